# revision 3
# baseline (speedup 1.0000x reference)
"""LipschitzRNN Trainium2 kernel.

Math (per reference):
    bA = 0.5*exp(-bA_z^2)+0.5 ; bW likewise
    A = (1-bA)(MA+MA.T) + bA(MA-MA.T) - YA*I
    C = (1-bA)(MW+MW.T) + bW(MW-MW.T) - YW*I
    X_{t+1} = X_t + STEP*(A@X_t + tanh(C@X_t + by))   (column-state X: [n, bs])
    out[b, t, :] = X_t[:, b]

Device strategy (8-way batch data-parallel, b=32/core, no collectives):
  - State rescaled: z = X/STEP, kept ONLY in fp16.  The recurrence becomes
        z' = z + (STEP*A)@z + tanh((STEP*C)@z + by)
    so every weight matrix is STEP-scaled (tiny entries, fp16-safe in
    RELATIVE terms) and the tanh term enters unscaled.  The identity carry
    I@z is folded into the U-matmuls with an exact fp16 identity weight,
    so no fp32 master state is needed; fp16 state rounding random-walks to
    ~6e-3 relative over 511 steps (budget 2e-2).
  - Two batch groups of 16 columns run anti-phased to hide the serial
    chain  V-matmuls -> tanh -> stt -> next V-matmuls  (~1.1us per step
    per group).  Per group-step:
      * Vbank[128, 2x16] PSUM: 2 bias-matmuls (k=1, by row x ones) +
        4 (STEP*C).T matmuls              (bias folded => ONE tanh instr)
      * Ubank[128, 2x16] PSUM: 2 identity + 4 (STEP*A).T matmuls
      * ONE activation: T = tanh(Vbank) -> SBUF fp16
      * ONE stt: z' = Ubank + T -> fp16, written directly into the DMA
        staging slot (doubles as next step's matmul input)
  - Output: z' blocks [128, 16 steps, 32] DMA'd raw every 16 steps
    (1KB/descriptor), layout [g, p, t, c*16+b].  Host transposes to
    [b, t, n] and multiplies by STEP.  No PE transposes, no staging
    copies.
"""

import numpy as np

N = 256
BS = 256
TMAX = 512
STEP = 0.01
YA = 0.001
YW = 0.001
NCORES = 8
BLOC = BS // NCORES      # 32 batch cols per core
NG = 2                   # anti-phased batch groups per core
GB = BLOC // NG          # 16 batch cols per group
GRP = 16                 # output steps per DMA block
NSTEPS = TMAX - 1        # 511

LAST_RESULT = None  # BassKernelResults of the most recent run (for test harness)


def _build(n_steps):
    from concourse import bacc, tile
    import concourse.mybir as mybir
    from concourse.masks import make_identity

    F32 = mybir.dt.float32
    F16 = mybir.dt.float16
    AF = mybir.ActivationFunctionType
    ALU = mybir.AluOpType

    nc = bacc.Bacc("TRN2", target_bir_lowering=False, debug=False,
                   num_devices=NCORES)

    WC = nc.dram_tensor("WC", [N, N], F16, kind="ExternalInput")   # (STEP*C).T
    WA = nc.dram_tensor("WA", [N, N], F16, kind="ExternalInput")   # (STEP*A).T
    BY = nc.dram_tensor("BY", [2, 128], F16, kind="ExternalInput")
    ONES = nc.dram_tensor("ONES", [1, GB], F16, kind="ExternalInput")
    Z0 = nc.dram_tensor("Z0", [NG, 128, 2 * GB], F16, kind="ExternalInput")
    OUT = nc.dram_tensor("OUT", [NG, 128, n_steps, 2 * GB], F16,
                         kind="ExternalOutput")

    nblocks = (n_steps + GRP - 1) // GRP

    with tile.TileContext(nc) as tc:
        with (
            tc.tile_pool(name="consts", bufs=1) as consts,
            tc.tile_pool(name="tp0", bufs=2) as tp0,
            tc.tile_pool(name="tp1", bufs=2) as tp1,
            tc.tile_pool(name="stg0", bufs=2) as stgp0,
            tc.tile_pool(name="stg1", bufs=2) as stgp1,
            tc.tile_pool(name="psv0", bufs=2, space="PSUM") as psv0,
            tc.tile_pool(name="psv1", bufs=2, space="PSUM") as psv1,
            tc.tile_pool(name="psu0", bufs=2, space="PSUM") as psu0,
            tc.tile_pool(name="psu1", bufs=2, space="PSUM") as psu1,
        ):
            # ---- constants / initial state ----
            wc = [[consts.tile([128, 128], F16, tag=f"wc{k}{m}", name=f"wc{k}{m}")
                   for m in range(2)] for k in range(2)]
            wa = [[consts.tile([128, 128], F16, tag=f"wa{k}{m}", name=f"wa{k}{m}")
                   for m in range(2)] for k in range(2)]
            for k in range(2):
                for m in range(2):
                    nc.sync.dma_start(
                        wc[k][m][:], WC[128 * k:128 * (k + 1), 128 * m:128 * (m + 1)])
                    nc.sync.dma_start(
                        wa[k][m][:], WA[128 * k:128 * (k + 1), 128 * m:128 * (m + 1)])
            by = [consts.tile([1, 128], F16, tag=f"by{m}", name=f"by{m}") for m in range(2)]
            nc.sync.dma_start(by[0][:], BY[0:1, :])
            nc.sync.dma_start(by[1][:], BY[1:2, :])
            ones = consts.tile([1, GB], F16, tag="ones", name="ones")
            nc.sync.dma_start(ones[:], ONES[:, :])
            ident_f32 = consts.tile([128, 128], F32, tag="ident_f32", name="ident_f32")
            make_identity(nc, ident_f32[:])
            ident = consts.tile([128, 128], F16, tag="ident", name="ident")
            nc.vector.tensor_copy(ident[:], ident_f32[:])

            z0t = [consts.tile([128, 2 * GB], F16, tag=f"z0_{g}", name=f"z0_{g}")
                   for g in range(NG)]
            for g in range(NG):
                nc.sync.dma_start(z0t[g][:], Z0[g, :, :])

            tpools = [tp0, tp1]
            stgpools = [stgp0, stgp1]
            pvpools = [psv0, psv1]
            pupools = [psu0, psu1]
            z = [z0t[g][:] for g in range(NG)]   # APs of current state
            stg = [None] * NG

            # ---- recurrence ----
            for t in range(1, n_steps + 1):
                s = (t - 1) % GRP               # slot within DMA block
                t0 = t - 1 - s                  # first step index of block
                blk = min(GRP, n_steps - t0)    # block size (last is partial)
                for g in range(NG):
                    # V = (STEP*C)@z + by   (bias via k=1 matmuls)
                    pv = pvpools[g].tile([128, 2 * GB], F32, tag=f"pv{g}",
                                         name=f"pv{g}")
                    pu = pupools[g].tile([128, 2 * GB], F32, tag=f"pu{g}",
                                         name=f"pu{g}")
                    for m in range(2):
                        o = pv[:, m * GB:(m + 1) * GB]
                        nc.tensor.matmul(o, by[m][:], ones[:],
                                         start=True, stop=False)
                        nc.tensor.matmul(o, wc[0][m][:], z[g][:, 0:GB],
                                         start=False, stop=False)
                        nc.tensor.matmul(o, wc[1][m][:], z[g][:, GB:2 * GB],
                                         start=False, stop=True)
                    # U = z + (STEP*A)@z   (identity carry, exact in fp16)
                    for m in range(2):
                        o = pu[:, m * GB:(m + 1) * GB]
                        nc.tensor.matmul(o, ident[:], z[g][:, m * GB:(m + 1) * GB],
                                         start=True, stop=False)
                        nc.tensor.matmul(o, wa[0][m][:], z[g][:, 0:GB],
                                         start=False, stop=False)
                        nc.tensor.matmul(o, wa[1][m][:], z[g][:, GB:2 * GB],
                                         start=False, stop=True)

                    # T = tanh(V)  (single instr, both m-chunks)
                    tt = tpools[g].tile([128, 2 * GB], F16, tag=f"tt{g}",
                                        name=f"tt{g}")
                    nc.scalar.activation(tt[:], pv[:], AF.Tanh)

                    # z' = U + T  -> fp16, straight into the DMA staging slot
                    if s == 0:
                        stg[g] = stgpools[g].tile([128, GRP, 2 * GB], F16,
                                                  tag=f"stg{g}", name=f"stg{g}")
                    zn = stg[g][:, s, :]
                    nc.vector.scalar_tensor_tensor(
                        zn, pu[:], 1.0, tt[:], op0=ALU.mult, op1=ALU.add)
                    z[g] = zn

                    if s == blk - 1:
                        nc.sync.dma_start(OUT[g, :, t0:t0 + blk, :],
                                          stg[g][:, 0:blk, :])
    nc.compile()
    return nc


def kernel(X0, MA, MW, bA_z, bW_z, by_w):
    global LAST_RESULT
    from concourse.bass_utils import run_bass_kernel_spmd

    X0 = np.asarray(X0, dtype=np.float32)
    MA = np.asarray(MA, dtype=np.float32)
    MW = np.asarray(MW, dtype=np.float32)
    bA_z = np.asarray(bA_z, dtype=np.float32)
    bW_z = np.asarray(bW_z, dtype=np.float32)
    by_w = np.asarray(by_w, dtype=np.float32)

    # host-side weight prep (f64/f32 math, STEP-scaled, then fp16)
    bA = np.float32(0.5) * np.exp(-bA_z[0, 0] * bA_z[0, 0]) + np.float32(0.5)
    bW = np.float32(0.5) * np.exp(-bW_z[0, 0] * bW_z[0, 0]) + np.float32(0.5)
    I = np.eye(N, dtype=np.float32)
    A = (1 - bA) * (MA + MA.T) + bA * (MA - MA.T) - np.float32(YA) * I
    C = (1 - bA) * (MW + MW.T) + bW * (MW - MW.T) - np.float32(YW) * I
    WCh = np.ascontiguousarray((np.float32(STEP) * C).T).astype(np.float16)
    WAh = np.ascontiguousarray((np.float32(STEP) * A).T).astype(np.float16)
    BYh = np.ascontiguousarray(by_w.reshape(2, 128)).astype(np.float16)
    ONESh = np.ones((1, GB), dtype=np.float16)

    in_maps = []
    for i in range(NCORES):
        Xc = X0[i * BLOC:(i + 1) * BLOC, :] / np.float32(STEP)  # [32, 256]
        Z0h = np.empty((NG, 128, 2 * GB), dtype=np.float16)
        for g in range(NG):
            rows = Xc[g * GB:(g + 1) * GB]                       # [16, 256]
            Z0h[g, :, 0:GB] = rows[:, 0:128].T
            Z0h[g, :, GB:2 * GB] = rows[:, 128:256].T
        in_maps.append({
            "WC": WCh, "WA": WAh, "BY": BYh, "ONES": ONESh, "Z0": Z0h,
        })

    nc = _build(NSTEPS)
    res = run_bass_kernel_spmd(nc, in_maps, core_ids=list(range(NCORES)))
    LAST_RESULT = res

    out = np.empty((BS, TMAX, N), dtype=np.float32)
    out[:, 0, :] = X0
    for i in range(NCORES):
        O = res.results[i]["OUT"]                   # [NG, 128, 511, 2*GB] f16
        O = np.asarray(O).reshape(NG, 128, NSTEPS, 2, GB)
        # [g, p, t, c, j] -> [g, j, t, c, p] -> [32, 511, 256]
        blockX = O.transpose(0, 4, 2, 3, 1).reshape(BLOC, NSTEPS, N)
        out[i * BLOC:(i + 1) * BLOC, 1:, :] = (
            blockX.astype(np.float32) * np.float32(STEP))
    return out


if __name__ == "__main__":
    rng = np.random.default_rng(0)
    inputs = {
        "X0": rng.standard_normal((BS, N), dtype=np.float32),
        "MA": rng.standard_normal((N, N), dtype=np.float32) / 16,
        "MW": rng.standard_normal((N, N), dtype=np.float32) / 16,
        "bA_z": np.full((1, 1), 0.65, dtype=np.float32),
        "bW_z": np.full((1, 1), 0.65, dtype=np.float32),
        "by_w": rng.standard_normal((N, 1), dtype=np.float32) / 100,
    }
    out = kernel(**inputs)
    print("out", out.shape, out.dtype, np.abs(out).max())


# revision 6
# speedup vs baseline: 1.0708x; 1.0708x over previous
"""LipschitzRNN Trainium2 kernel.

Math (per reference):
    bA = 0.5*exp(-bA_z^2)+0.5 ; bW likewise
    A = (1-bA)(MA+MA.T) + bA(MA-MA.T) - YA*I
    C = (1-bA)(MW+MW.T) + bW(MW-MW.T) - YW*I
    X_{t+1} = X_t + STEP*(A@X_t + tanh(C@X_t + by))   (column-state X: [n, bs])
    out[b, t, :] = X_t[:, b]

Device strategy (8-way batch data-parallel, b=32/core, no collectives):
  - State rescaled: z = X/STEP, kept ONLY in fp16.  The recurrence becomes
        z' = z + (STEP*A)@z + tanh((STEP*C)@z + by)
    so every weight matrix is STEP-scaled (tiny entries, fp16-safe in
    RELATIVE terms) and the tanh term enters unscaled.  The identity carry
    I@z is folded into the U-matmuls with an exact fp16 identity weight,
    so no fp32 master state is needed; fp16 state rounding random-walks to
    ~6e-3 relative over 511 steps (budget 2e-2).
  - Two batch groups of 16 columns run anti-phased to hide the serial
    chain  V-matmuls -> tanh -> stt -> next V-matmuls  (~1.1us per step
    per group).  Per group-step:
      * Vbank[128, 2x16] PSUM: 2 bias-matmuls (k=1, by row x ones) +
        4 (STEP*C).T matmuls              (bias folded => ONE tanh instr)
      * Ubank[128, 2x16] PSUM: 2 identity + 4 (STEP*A).T matmuls
      * ONE activation: T = tanh(Vbank) -> SBUF fp16
      * ONE stt: z' = Ubank + T -> fp16, written directly into the DMA
        staging slot (doubles as next step's matmul input)
  - Output: z' blocks [128, 16 steps, 32] DMA'd raw every 16 steps
    (1KB/descriptor), layout [g, p, t, c*16+b].  Host transposes to
    [b, t, n] and multiplies by STEP.  No PE transposes, no staging
    copies.
"""

import numpy as np

N = 256
BS = 256
TMAX = 512
STEP = 0.01
YA = 0.001
YW = 0.001
NCORES = 8
BLOC = BS // NCORES      # 32 batch cols per core
NG = 2                   # anti-phased batch groups per core
GB = BLOC // NG          # 16 batch cols per group
GRP = 16                 # output steps per DMA block
NSTEPS = TMAX - 1        # 511

LAST_RESULT = None  # BassKernelResults of the most recent run (for test harness)


def _build(n_steps):
    from concourse import bacc, tile
    import concourse.mybir as mybir
    from concourse.masks import make_identity

    F32 = mybir.dt.float32
    F16 = mybir.dt.float16
    AF = mybir.ActivationFunctionType
    ALU = mybir.AluOpType

    nc = bacc.Bacc("TRN2", target_bir_lowering=False, debug=False,
                   num_devices=NCORES)

    WC = nc.dram_tensor("WC", [N, N], F16, kind="ExternalInput")   # (STEP*C).T
    WA = nc.dram_tensor("WA", [N, N], F16, kind="ExternalInput")   # (STEP*A).T
    BY = nc.dram_tensor("BY", [2, 128], F16, kind="ExternalInput")
    ONES = nc.dram_tensor("ONES", [1, GB], F16, kind="ExternalInput")
    Z0 = nc.dram_tensor("Z0", [NG, 128, 2 * GB], F16, kind="ExternalInput")
    OUT = nc.dram_tensor("OUT", [NG, 128, n_steps, 2 * GB], F16,
                         kind="ExternalOutput")

    nblocks = (n_steps + GRP - 1) // GRP

    with tile.TileContext(nc) as tc:
        with (
            tc.tile_pool(name="consts", bufs=1) as consts,
            tc.tile_pool(name="tp0", bufs=3) as tp0,
            tc.tile_pool(name="tp1", bufs=3) as tp1,
            tc.tile_pool(name="stg0", bufs=2) as stgp0,
            tc.tile_pool(name="stg1", bufs=2) as stgp1,
            tc.tile_pool(name="ps0", bufs=4, space="PSUM") as ps0,
            tc.tile_pool(name="ps1", bufs=4, space="PSUM") as ps1,
        ):
            # ---- constants / initial state ----
            wc = [[consts.tile([128, 128], F16, tag=f"wc{k}{m}", name=f"wc{k}{m}")
                   for m in range(2)] for k in range(2)]
            wa = [[consts.tile([128, 128], F16, tag=f"wa{k}{m}", name=f"wa{k}{m}")
                   for m in range(2)] for k in range(2)]
            for k in range(2):
                for m in range(2):
                    nc.sync.dma_start(
                        wc[k][m][:], WC[128 * k:128 * (k + 1), 128 * m:128 * (m + 1)])
                    nc.sync.dma_start(
                        wa[k][m][:], WA[128 * k:128 * (k + 1), 128 * m:128 * (m + 1)])
            by = [consts.tile([1, 128], F16, tag=f"by{m}", name=f"by{m}") for m in range(2)]
            nc.sync.dma_start(by[0][:], BY[0:1, :])
            nc.sync.dma_start(by[1][:], BY[1:2, :])
            ones = consts.tile([1, GB], F16, tag="ones", name="ones")
            nc.sync.dma_start(ones[:], ONES[:, :])
            ident_f32 = consts.tile([128, 128], F32, tag="ident_f32", name="ident_f32")
            make_identity(nc, ident_f32[:])
            ident = consts.tile([128, 128], F16, tag="ident", name="ident")
            nc.vector.tensor_copy(ident[:], ident_f32[:])

            z0t = [consts.tile([128, 2 * GB], F16, tag=f"z0_{g}", name=f"z0_{g}")
                   for g in range(NG)]
            for g in range(NG):
                nc.sync.dma_start(z0t[g][:], Z0[g, :, :])

            tpools = [tp0, tp1]
            stgpools = [stgp0, stgp1]
            pspools = [ps0, ps1]
            z = [z0t[g][:] for g in range(NG)]   # APs of current state
            stg = [None] * NG

            # ---- recurrence ----
            for t in range(1, n_steps + 1):
                s = (t - 1) % GRP               # slot within DMA block
                t0 = t - 1 - s                  # first step index of block
                blk = min(GRP, n_steps - t0)    # block size (last is partial)
                for g in range(NG):
                    # V = (STEP*C)@z + by   (bias via k=1 matmuls)
                    pvu = pspools[g].tile([128, 4 * GB], F32, tag=f"pvu{g}",
                                          name=f"pvu{g}")
                    pv = pvu[:, 0:2 * GB]
                    pu = pvu[:, 2 * GB:4 * GB]
                    for m in range(2):
                        o = pv[:, m * GB:(m + 1) * GB]
                        nc.tensor.matmul(o, by[m][:], ones[:],
                                         start=True, stop=False)
                        nc.tensor.matmul(o, wc[0][m][:], z[g][:, 0:GB],
                                         start=False, stop=False)
                        nc.tensor.matmul(o, wc[1][m][:], z[g][:, GB:2 * GB],
                                         start=False, stop=True)
                    # U = z + (STEP*A)@z   (identity carry, exact in fp16;
                    # one ident matmul covers both m-chunks: I@[z0|z1])
                    nc.tensor.matmul(pu[:, 0:2 * GB], ident[:], z[g][:, 0:2 * GB],
                                     start=True, stop=False)
                    for m in range(2):
                        o = pu[:, m * GB:(m + 1) * GB]
                        nc.tensor.matmul(o, wa[0][m][:], z[g][:, 0:GB],
                                         start=False, stop=False)
                        nc.tensor.matmul(o, wa[1][m][:], z[g][:, GB:2 * GB],
                                         start=False, stop=True)

                    # T = tanh(V)  (single instr, both m-chunks)
                    tt = tpools[g].tile([128, 2 * GB], F16, tag=f"tt{g}",
                                        name=f"tt{g}")
                    nc.scalar.activation(tt[:], pv[:, :], AF.Tanh)

                    # z' = U + T  -> fp16, straight into the DMA staging slot
                    if s == 0:
                        stg[g] = stgpools[g].tile([128, GRP, 2 * GB], F16,
                                                  tag=f"stg{g}", name=f"stg{g}")
                    zn = stg[g][:, s, :]
                    nc.vector.scalar_tensor_tensor(
                        zn, pu[:, :], 1.0, tt[:], op0=ALU.mult, op1=ALU.add)
                    z[g] = zn

                    if s == blk - 1:
                        nc.sync.dma_start(OUT[g, :, t0:t0 + blk, :],
                                          stg[g][:, 0:blk, :])
    nc.compile()
    return nc


def kernel(X0, MA, MW, bA_z, bW_z, by_w):
    global LAST_RESULT
    from concourse.bass_utils import run_bass_kernel_spmd

    X0 = np.asarray(X0, dtype=np.float32)
    MA = np.asarray(MA, dtype=np.float32)
    MW = np.asarray(MW, dtype=np.float32)
    bA_z = np.asarray(bA_z, dtype=np.float32)
    bW_z = np.asarray(bW_z, dtype=np.float32)
    by_w = np.asarray(by_w, dtype=np.float32)

    # host-side weight prep (f64/f32 math, STEP-scaled, then fp16)
    bA = np.float32(0.5) * np.exp(-bA_z[0, 0] * bA_z[0, 0]) + np.float32(0.5)
    bW = np.float32(0.5) * np.exp(-bW_z[0, 0] * bW_z[0, 0]) + np.float32(0.5)
    I = np.eye(N, dtype=np.float32)
    A = (1 - bA) * (MA + MA.T) + bA * (MA - MA.T) - np.float32(YA) * I
    C = (1 - bA) * (MW + MW.T) + bW * (MW - MW.T) - np.float32(YW) * I
    WCh = np.ascontiguousarray((np.float32(STEP) * C).T).astype(np.float16)
    WAh = np.ascontiguousarray((np.float32(STEP) * A).T).astype(np.float16)
    BYh = np.ascontiguousarray(by_w.reshape(2, 128)).astype(np.float16)
    ONESh = np.ones((1, GB), dtype=np.float16)

    in_maps = []
    for i in range(NCORES):
        Xc = X0[i * BLOC:(i + 1) * BLOC, :] / np.float32(STEP)  # [32, 256]
        Z0h = np.empty((NG, 128, 2 * GB), dtype=np.float16)
        for g in range(NG):
            rows = Xc[g * GB:(g + 1) * GB]                       # [16, 256]
            Z0h[g, :, 0:GB] = rows[:, 0:128].T
            Z0h[g, :, GB:2 * GB] = rows[:, 128:256].T
        in_maps.append({
            "WC": WCh, "WA": WAh, "BY": BYh, "ONES": ONESh, "Z0": Z0h,
        })

    nc = _build(NSTEPS)
    res = run_bass_kernel_spmd(nc, in_maps, core_ids=list(range(NCORES)))
    LAST_RESULT = res

    out = np.empty((BS, TMAX, N), dtype=np.float32)
    out[:, 0, :] = X0
    for i in range(NCORES):
        O = res.results[i]["OUT"]                   # [NG, 128, 511, 2*GB] f16
        O = np.asarray(O).reshape(NG, 128, NSTEPS, 2, GB)
        # [g, p, t, c, j] -> [g, j, t, c, p] -> [32, 511, 256]
        blockX = O.transpose(0, 4, 2, 3, 1).reshape(BLOC, NSTEPS, N)
        out[i * BLOC:(i + 1) * BLOC, 1:, :] = (
            blockX.astype(np.float32) * np.float32(STEP))
    return out


if __name__ == "__main__":
    rng = np.random.default_rng(0)
    inputs = {
        "X0": rng.standard_normal((BS, N), dtype=np.float32),
        "MA": rng.standard_normal((N, N), dtype=np.float32) / 16,
        "MW": rng.standard_normal((N, N), dtype=np.float32) / 16,
        "bA_z": np.full((1, 1), 0.65, dtype=np.float32),
        "bW_z": np.full((1, 1), 0.65, dtype=np.float32),
        "by_w": rng.standard_normal((N, 1), dtype=np.float32) / 100,
    }
    out = kernel(**inputs)
    print("out", out.shape, out.dtype, np.abs(out).max())


# revision 8
# speedup vs baseline: 1.6264x; 1.5189x over previous
"""LipschitzRNN Trainium2 kernel — correction-chain design.

Math (per reference):
    A = (1-bA)(MA+MA.T) + bA(MA-MA.T) - YA*I ; C likewise with bW mix
    X_{t+1} = X_t + STEP*(A@X_t + tanh(C@X_t + by))
    out[b, t, :] = X_t[:, b]

Device strategy (8-way batch data-parallel, 32 cols/core, no collectives):
  State z = X/STEP in fp16 (STEP-scaled weights keep fp16 relative error;
  the tanh term enters unscaled; fp16 carry random-walks to ~9.5e-3 rel,
  budget 2e-2).  The serial per-step chain is the wall-clock limit
  (engines are mostly idle), so the tanh is taken OFF the chain by a
  first-order Taylor correction:

      Yhat_j = by + (Cs D)@z_{j-1}          (one step of slack; D = I+Ah)
      That_j = tanh(Yhat_j),  S_j = 1 - That_j^2
      c_j    = (Cs@That_{j-1}) * S_j        (first-order delta correction)
      z_j    = z_{j-1} + Ah@z_{j-1} + That_{j-1} [+ c_{j-4}, injected late]

  with Ah = STEP*A, Cs = STEP*C (fp16), CsD = f16(STEP*C@(I+STEP*A)).
  The only true serial chain is  z -> {I,Ah}-matmuls -> CAST -> z
  (~700ns); tanh/Square/S/c ride parallel tracks with >=1 step slack, and
  the tiny correction (|c|~0.05 in z units ~ 1e-4 of X) enters 3 steps
  late, which the numpy model shows costs nothing (rel err 9.53e-3).
  Bias is folded into the Yhat bank with padded-128 "bias weight"
  matmuls (row 0 = by chunk) against a constant e0 tile, so a single
  [128,64] tanh instruction needs no per-partition bias and the PE never
  switches tile configs (k=1 matmuls cost ~100ns reconfig stalls).
  Output: z blocks [128, 16 steps, 64] DMA'd raw every 16 steps
  (2KB/descriptor); host transposes [p,t,c*32+b] -> [b,t,n] and scales
  by STEP.  No PE transposes, no staging copies.
"""

import numpy as np

N = 256
BS = 256
TMAX = 512
STEP = 0.01
YA = 0.001
YW = 0.001
NCORES = 8
BLOC = BS // NCORES      # 32 batch cols per core
GRP = 16                 # output steps per DMA block
NSTEPS = TMAX - 1        # 511
CLAG = 4                 # c_{j-CLAG} injected into zb_j

LAST_RESULT = None  # BassKernelResults of the most recent run (for test harness)


def _build(n_steps):
    from concourse import bacc, tile
    import concourse.mybir as mybir
    from concourse.masks import make_identity

    F32 = mybir.dt.float32
    F16 = mybir.dt.float16
    AF = mybir.ActivationFunctionType
    ALU = mybir.AluOpType

    nc = bacc.Bacc("TRN2", target_bir_lowering=False, debug=False,
                   num_devices=NCORES)

    WA = nc.dram_tensor("WA", [N, N], F16, kind="ExternalInput")    # (STEP*A).T
    WCS = nc.dram_tensor("WCS", [N, N], F16, kind="ExternalInput")  # (STEP*C).T
    WCD = nc.dram_tensor("WCD", [N, N], F16, kind="ExternalInput")  # (STEP*C*D).T
    BP = nc.dram_tensor("BP", [2, 128, 128], F16, kind="ExternalInput")
    E0 = nc.dram_tensor("E0", [128, BLOC], F16, kind="ExternalInput")
    Z0 = nc.dram_tensor("Z0", [128, 2 * BLOC], F16, kind="ExternalInput")
    T0 = nc.dram_tensor("T0", [128, 2 * BLOC], F16, kind="ExternalInput")
    OUT = nc.dram_tensor("OUT", [128, n_steps, 2 * BLOC], F16,
                         kind="ExternalOutput")

    W = 2 * BLOC  # 64: working tile width (2 n-chunks x 32 batch)

    with tile.TileContext(nc) as tc:
        with (
            tc.tile_pool(name="consts", bufs=1) as consts,
            tc.tile_pool(name="tpool", bufs=3) as tpool,
            tc.tile_pool(name="qpool", bufs=3) as qpool,
            tc.tile_pool(name="spool", bufs=2) as spool,
            tc.tile_pool(name="cpool", bufs=CLAG + 3) as cpool,
            tc.tile_pool(name="stg", bufs=2) as stgp,
            tc.tile_pool(name="zb", bufs=2, space="PSUM") as zbp,
            tc.tile_pool(name="yb", bufs=2, space="PSUM") as ybp,
            tc.tile_pool(name="db", bufs=3, space="PSUM") as dbp,
        ):
            # ---- constants / initial state ----
            wa = [[consts.tile([128, 128], F16, tag=f"wa{k}{m}", name=f"wa{k}{m}")
                   for m in range(2)] for k in range(2)]
            wcs = [[consts.tile([128, 128], F16, tag=f"wcs{k}{m}", name=f"wcs{k}{m}")
                    for m in range(2)] for k in range(2)]
            wcd = [[consts.tile([128, 128], F16, tag=f"wcd{k}{m}", name=f"wcd{k}{m}")
                    for m in range(2)] for k in range(2)]
            for k in range(2):
                for m in range(2):
                    sl = (slice(128 * k, 128 * (k + 1)), slice(128 * m, 128 * (m + 1)))
                    nc.sync.dma_start(wa[k][m][:], WA[sl[0], sl[1]])
                    nc.sync.dma_start(wcs[k][m][:], WCS[sl[0], sl[1]])
                    nc.sync.dma_start(wcd[k][m][:], WCD[sl[0], sl[1]])
            bp = [consts.tile([128, 128], F16, tag=f"bp{m}", name=f"bp{m}")
                  for m in range(2)]
            nc.sync.dma_start(bp[0][:], BP[0])
            nc.sync.dma_start(bp[1][:], BP[1])
            e0 = consts.tile([128, BLOC], F16, tag="e0", name="e0")
            nc.sync.dma_start(e0[:], E0[:, :])
            ident_f32 = consts.tile([128, 128], F32, tag="idf", name="idf")
            make_identity(nc, ident_f32[:])
            ident = consts.tile([128, 128], F16, tag="ident", name="ident")
            nc.vector.tensor_copy(ident[:], ident_f32[:])
            z0t = consts.tile([128, W], F16, tag="z0t", name="z0t")
            nc.sync.dma_start(z0t[:], Z0[:, :])
            t0t = consts.tile([128, W], F16, tag="t0t", name="t0t")
            nc.sync.dma_start(t0t[:], T0[:, :])

            def ybank_mms(yb, zsrc):
                # Yhat bank: bias (padded weights vs e0) + CsD @ z
                nc.tensor.matmul(yb[:, 0:BLOC], bp[0][:], e0[:],
                                 start=True, stop=False)
                nc.tensor.matmul(yb[:, BLOC:W], bp[1][:], e0[:],
                                 start=True, stop=False)
                for m in range(2):
                    o = yb[:, m * BLOC:(m + 1) * BLOC]
                    nc.tensor.matmul(o, wcd[0][m][:], zsrc[:, 0:BLOC],
                                     start=False, stop=False)
                    nc.tensor.matmul(o, wcd[1][m][:], zsrc[:, BLOC:W],
                                     start=False, stop=True)

            # ---- pre-loop: Yhat_1 / That_1 / Q_1 from z0 ----
            yb = ybp.tile([128, W], F32, tag="yb", name="yb")
            ybank_mms(yb, z0t)
            t_j = tpool.tile([128, W], F16, tag="tt", name="tt")  # That_1
            nc.scalar.activation(t_j[:], yb[:], AF.Tanh)
            q_j = qpool.tile([128, W], F16, tag="qq", name="qq")  # Q_1
            nc.scalar.activation(q_j[:], t_j[:], AF.Square)

            t_jm1 = t0t             # That_{j-1} (That_0 := T0)
            q_jm1 = None            # Q_{j-1}
            db_jm1 = None           # delta-hat bank from prev iter
            cql = []                # pending c tiles (FIFO)
            z = z0t[:]
            stg = None

            for j in range(1, n_steps + 1):
                s = (j - 1) % GRP
                t0i = j - 1 - s
                blk = min(GRP, n_steps - t0i)

                # ---- z bank: I@z + Ah@z [+ I@c_{j-CLAG}] + I@That_{j-1} ----
                zb = zbp.tile([128, W], F32, tag="zbk", name="zbk")
                nc.tensor.matmul(zb[:, 0:W], ident[:], z[:, 0:W],
                                 start=True, stop=False, skip_group_check=True)
                for m in range(2):
                    o = zb[:, m * BLOC:(m + 1) * BLOC]
                    nc.tensor.matmul(o, wa[0][m][:], z[:, 0:BLOC],
                                     start=False, stop=False,
                                     skip_group_check=True)
                    nc.tensor.matmul(o, wa[1][m][:], z[:, BLOC:W],
                                     start=False, stop=False,
                                     skip_group_check=True)
                if j > CLAG:
                    cinj = cql.pop(0)
                    nc.tensor.matmul(zb[:, 0:W], ident[:], cinj[:, 0:W],
                                     start=False, stop=False,
                                     skip_group_check=True)
                nc.tensor.matmul(zb[:, 0:W], ident[:], t_jm1[:, 0:W],
                                 start=False, stop=True, skip_group_check=True)

                # ---- delta-hat bank: Cs @ That_{j-1} ----
                db = dbp.tile([128, W], F32, tag="dbk", name="dbk")
                for m in range(2):
                    o = db[:, m * BLOC:(m + 1) * BLOC]
                    nc.tensor.matmul(o, wcs[0][m][:], t_jm1[:, 0:BLOC],
                                     start=True, stop=False)
                    nc.tensor.matmul(o, wcs[1][m][:], t_jm1[:, BLOC:W],
                                     start=False, stop=True)

                # ---- CAST: z_j = zb -> fp16, straight into DMA staging ----
                if s == 0:
                    stg = stgp.tile([128, GRP, W], F16, tag="stg", name="stg")
                zn = stg[:, s, :]
                nc.vector.tensor_copy(zn, zb[:])

                # ---- Yhat_{j+1} bank + That_{j+1} + Q_{j+1} (uses z_j) ----
                t_jp1 = q_jp1 = None
                if j < n_steps:
                    yb = ybp.tile([128, W], F32, tag="yb", name="yb")
                    ybank_mms(yb, zn)
                    t_jp1 = tpool.tile([128, W], F16, tag="tt", name="tt")
                    nc.scalar.activation(t_jp1[:], yb[:], AF.Tanh)
                    q_jp1 = qpool.tile([128, W], F16, tag="qq", name="qq")
                    nc.scalar.activation(q_jp1[:], t_jp1[:], AF.Square)

                # ---- S_{j-1} and c_{j-1} (one iteration late, off-chain) ----
                if 2 <= j <= n_steps - CLAG + 1:
                    ss = spool.tile([128, W], F16, tag="ss", name="ss")
                    nc.vector.tensor_scalar(ss[:], q_jm1[:], -1.0, 1.0,
                                            op0=ALU.mult, op1=ALU.add)
                    cc = cpool.tile([128, W], F16, tag="cc", name="cc")
                    nc.vector.scalar_tensor_tensor(
                        cc[:], db_jm1[:], 1.0, ss[:],
                        op0=ALU.mult, op1=ALU.mult)
                    cql.append(cc)

                if s == blk - 1:
                    nc.sync.dma_start(OUT[:, t0i:t0i + blk, :],
                                      stg[:, 0:blk, :])

                z = zn
                t_jm1 = t_j
                t_j = t_jp1
                q_jm1 = q_j
                q_j = q_jp1
                db_jm1 = db
    nc.compile()
    return nc


def kernel(X0, MA, MW, bA_z, bW_z, by_w):
    global LAST_RESULT
    from concourse.bass_utils import run_bass_kernel_spmd

    X0 = np.asarray(X0, dtype=np.float32)
    MA = np.asarray(MA, dtype=np.float32)
    MW = np.asarray(MW, dtype=np.float32)
    bA_z = np.asarray(bA_z, dtype=np.float32)
    bW_z = np.asarray(bW_z, dtype=np.float32)
    by_w = np.asarray(by_w, dtype=np.float32)

    bA = np.float32(0.5) * np.exp(-bA_z[0, 0] * bA_z[0, 0]) + np.float32(0.5)
    bW = np.float32(0.5) * np.exp(-bW_z[0, 0] * bW_z[0, 0]) + np.float32(0.5)
    I = np.eye(N, dtype=np.float32)
    A = (1 - bA) * (MA + MA.T) + bA * (MA - MA.T) - np.float32(YA) * I
    C = (1 - bA) * (MW + MW.T) + bW * (MW - MW.T) - np.float32(YW) * I

    f16 = lambda x: x.astype(np.float16).astype(np.float32)
    Ah = f16(np.float32(STEP) * A)
    Cs = f16(np.float32(STEP) * C)
    CsD = f16((np.float32(STEP) * C) @ (I + np.float32(STEP) * A))
    byh = f16(by_w)

    WAh = np.ascontiguousarray(Ah.T).astype(np.float16)
    WCSh = np.ascontiguousarray(Cs.T).astype(np.float16)
    WCDh = np.ascontiguousarray(CsD.T).astype(np.float16)
    BPh = np.zeros((2, 128, 128), dtype=np.float16)
    BPh[0, 0, :] = byh[0:128, 0].astype(np.float16)
    BPh[1, 0, :] = byh[128:256, 0].astype(np.float16)
    E0h = np.zeros((128, BLOC), dtype=np.float16)
    E0h[0, :] = 1.0

    in_maps = []
    for i in range(NCORES):
        Xc = X0[i * BLOC:(i + 1) * BLOC, :] / np.float32(STEP)   # [32, 256]
        z0 = Xc.T.astype(np.float16)                              # [256, 32]
        z0f = z0.astype(np.float32)
        T0f = np.tanh(Cs @ z0f + byh).astype(np.float16)          # [256, 32]
        Z0h = np.empty((128, 2 * BLOC), dtype=np.float16)
        T0h = np.empty((128, 2 * BLOC), dtype=np.float16)
        for c in range(2):
            Z0h[:, c * BLOC:(c + 1) * BLOC] = z0[c * 128:(c + 1) * 128, :]
            T0h[:, c * BLOC:(c + 1) * BLOC] = T0f[c * 128:(c + 1) * 128, :]
        in_maps.append({
            "WA": WAh, "WCS": WCSh, "WCD": WCDh, "BP": BPh, "E0": E0h,
            "Z0": Z0h, "T0": T0h,
        })

    nc = _build(NSTEPS)
    res = run_bass_kernel_spmd(nc, in_maps, core_ids=list(range(NCORES)))
    LAST_RESULT = res

    out = np.empty((BS, TMAX, N), dtype=np.float32)
    out[:, 0, :] = X0
    for i in range(NCORES):
        O = np.asarray(res.results[i]["OUT"]).reshape(128, NSTEPS, 2, BLOC)
        # [p, t, c, b] -> [b, t, c, p] -> [32, 511, 256]
        blockX = O.transpose(3, 1, 2, 0).reshape(BLOC, NSTEPS, N)
        out[i * BLOC:(i + 1) * BLOC, 1:, :] = (
            blockX.astype(np.float32) * np.float32(STEP))
    return out


if __name__ == "__main__":
    rng = np.random.default_rng(0)
    inputs = {
        "X0": rng.standard_normal((BS, N), dtype=np.float32),
        "MA": rng.standard_normal((N, N), dtype=np.float32) / 16,
        "MW": rng.standard_normal((N, N), dtype=np.float32) / 16,
        "bA_z": np.full((1, 1), 0.65, dtype=np.float32),
        "bW_z": np.full((1, 1), 0.65, dtype=np.float32),
        "by_w": rng.standard_normal((N, 1), dtype=np.float32) / 100,
    }
    out = kernel(**inputs)
    print("out", out.shape, out.dtype, np.abs(out).max())


# revision 10
# speedup vs baseline: 1.7408x; 1.0703x over previous
"""LipschitzRNN Trainium2 kernel — correction-chain design.

Math (per reference):
    A = (1-bA)(MA+MA.T) + bA(MA-MA.T) - YA*I ; C likewise with bW mix
    X_{t+1} = X_t + STEP*(A@X_t + tanh(C@X_t + by))
    out[b, t, :] = X_t[:, b]

Device strategy (8-way batch data-parallel, 32 cols/core, no collectives):
  State z = X/STEP in fp16 (STEP-scaled weights keep fp16 relative error;
  the tanh term enters unscaled; fp16 carry random-walks to ~9.5e-3 rel,
  budget 2e-2).  The serial per-step chain is the wall-clock limit
  (engines are mostly idle), so the tanh is taken OFF the chain by a
  first-order Taylor correction:

      Yhat_j = by + (Cs D)@z_{j-1}          (one step of slack; D = I+Ah)
      That_j = tanh(Yhat_j),  S_j = 1 - That_j^2
      c_j    = (Cs@That_{j-1}) * S_j        (first-order delta correction)
      z_j    = z_{j-1} + Ah@z_{j-1} + That_{j-1} [+ c_{j-4}, injected late]

  with Ah = STEP*A, Cs = STEP*C (fp16), CsD = f16(STEP*C@(I+STEP*A)).
  The only true serial chain is  z -> {I,Ah}-matmuls -> CAST -> z
  (~700ns); tanh/Square/S/c ride parallel tracks with >=1 step slack, and
  the tiny correction (|c|~0.05 in z units ~ 1e-4 of X) enters 3 steps
  late, which the numpy model shows costs nothing (rel err 9.53e-3).
  Bias is folded into the Yhat bank with padded-128 "bias weight"
  matmuls (row 0 = by chunk) against a constant e0 tile, so a single
  [128,64] tanh instruction needs no per-partition bias and the PE never
  switches tile configs (k=1 matmuls cost ~100ns reconfig stalls).
  Output: z blocks [128, 16 steps, 64] DMA'd raw every 16 steps
  (2KB/descriptor); host transposes [p,t,c*32+b] -> [b,t,n] and scales
  by STEP.  No PE transposes, no staging copies.
"""

import numpy as np

N = 256
BS = 256
TMAX = 512
STEP = 0.01
YA = 0.001
YW = 0.001
NCORES = 8
BLOC = BS // NCORES      # 32 batch cols per core
GRP = 16                 # output steps per DMA block
NSTEPS = TMAX - 1        # 511
CLAG = 4                 # c_{j-CLAG} injected into zb_j

LAST_RESULT = None  # BassKernelResults of the most recent run (for test harness)


def _build(n_steps):
    from concourse import bacc, tile
    import concourse.mybir as mybir
    from concourse.masks import make_identity

    F32 = mybir.dt.float32
    F16 = mybir.dt.float16
    AF = mybir.ActivationFunctionType
    ALU = mybir.AluOpType

    nc = bacc.Bacc("TRN2", target_bir_lowering=False, debug=False,
                   num_devices=NCORES)

    WA = nc.dram_tensor("WA", [N, N], F16, kind="ExternalInput")    # (STEP*A).T
    WCS = nc.dram_tensor("WCS", [N, N], F16, kind="ExternalInput")  # (STEP*C).T
    WCD = nc.dram_tensor("WCD", [N, N], F16, kind="ExternalInput")  # (STEP*C*D).T
    BP = nc.dram_tensor("BP", [2, 128, 128], F16, kind="ExternalInput")
    E0 = nc.dram_tensor("E0", [128, BLOC], F16, kind="ExternalInput")
    Z0 = nc.dram_tensor("Z0", [128, 2 * BLOC], F16, kind="ExternalInput")
    T0 = nc.dram_tensor("T0", [128, 2 * BLOC], F16, kind="ExternalInput")
    OUT = nc.dram_tensor("OUT", [128, n_steps, 2 * BLOC], F16,
                         kind="ExternalOutput")

    W = 2 * BLOC  # 64: working tile width (2 n-chunks x 32 batch)

    with tile.TileContext(nc) as tc:
        with (
            tc.tile_pool(name="consts", bufs=1) as consts,
            tc.tile_pool(name="tpool", bufs=3) as tpool,
            tc.tile_pool(name="qpool", bufs=3) as qpool,
            tc.tile_pool(name="spool", bufs=2) as spool,
            tc.tile_pool(name="cpool", bufs=CLAG + 3) as cpool,
            tc.tile_pool(name="stg", bufs=2) as stgp,
            tc.tile_pool(name="zb", bufs=1, space="PSUM") as zbp,
            tc.tile_pool(name="yb", bufs=3, space="PSUM") as ybp,
            tc.tile_pool(name="db", bufs=3, space="PSUM") as dbp,
        ):
            # ---- constants / initial state ----
            wa = [[consts.tile([128, 128], F16, tag=f"wa{k}{m}", name=f"wa{k}{m}")
                   for m in range(2)] for k in range(2)]
            wcs = [[consts.tile([128, 128], F16, tag=f"wcs{k}{m}", name=f"wcs{k}{m}")
                    for m in range(2)] for k in range(2)]
            wcd = [[consts.tile([128, 128], F16, tag=f"wcd{k}{m}", name=f"wcd{k}{m}")
                    for m in range(2)] for k in range(2)]
            for k in range(2):
                for m in range(2):
                    sl = (slice(128 * k, 128 * (k + 1)), slice(128 * m, 128 * (m + 1)))
                    nc.sync.dma_start(wa[k][m][:], WA[sl[0], sl[1]])
                    nc.sync.dma_start(wcs[k][m][:], WCS[sl[0], sl[1]])
                    nc.sync.dma_start(wcd[k][m][:], WCD[sl[0], sl[1]])
            bp = [consts.tile([128, 128], F16, tag=f"bp{m}", name=f"bp{m}")
                  for m in range(2)]
            nc.sync.dma_start(bp[0][:], BP[0])
            nc.sync.dma_start(bp[1][:], BP[1])
            e0 = consts.tile([128, BLOC], F16, tag="e0", name="e0")
            nc.sync.dma_start(e0[:], E0[:, :])
            ident_f32 = consts.tile([128, 128], F32, tag="idf", name="idf")
            make_identity(nc, ident_f32[:])
            ident = consts.tile([128, 128], F16, tag="ident", name="ident")
            nc.vector.tensor_copy(ident[:], ident_f32[:])
            z0t = consts.tile([128, W], F16, tag="z0t", name="z0t")
            nc.sync.dma_start(z0t[:], Z0[:, :])
            t0t = consts.tile([128, W], F16, tag="t0t", name="t0t")
            nc.sync.dma_start(t0t[:], T0[:, :])

            def ybank_mms(yb, zsrc):
                # Yhat bank: bias (padded weights vs e0) + CsD @ z
                nc.tensor.matmul(yb[:, 0:BLOC], bp[0][:], e0[:],
                                 start=True, stop=False)
                nc.tensor.matmul(yb[:, BLOC:W], bp[1][:], e0[:],
                                 start=True, stop=False)
                for m in range(2):
                    o = yb[:, m * BLOC:(m + 1) * BLOC]
                    nc.tensor.matmul(o, wcd[0][m][:], zsrc[:, 0:BLOC],
                                     start=False, stop=False)
                    nc.tensor.matmul(o, wcd[1][m][:], zsrc[:, BLOC:W],
                                     start=False, stop=True)

            # ---- persistent f32 z-bank: init M = I@z0, then accumulate
            # Ah@z + That + c onto it forever (exact f32 carry) ----
            zb = zbp.tile([128, W], F32, tag="zbk", name="zbk")
            nc.tensor.matmul(zb[:, 0:W], ident[:], z0t[:, 0:W],
                             start=True, stop=False, skip_group_check=True)

            # ---- pre-loop: Yhat_1 / That_1 / Q_1 from z0 ----
            yb = ybp.tile([128, W], F32, tag="yb", name="yb")
            ybank_mms(yb, z0t)
            t_j = tpool.tile([128, W], F16, tag="tt", name="tt")  # That_1
            nc.scalar.activation(t_j[:], yb[:], AF.Tanh)
            q_j = qpool.tile([128, W], F16, tag="qq", name="qq")  # Q_1
            nc.scalar.activation(q_j[:], t_j[:], AF.Square)

            t_jm1 = t0t             # That_{j-1} (That_0 := T0)
            q_jm1 = None            # Q_{j-1}
            db_jm1 = None           # delta-hat bank from prev iter
            cql = []                # pending c tiles (FIFO)
            z = z0t[:]
            stg = None

            for j in range(1, n_steps + 1):
                s = (j - 1) % GRP
                t0i = j - 1 - s
                blk = min(GRP, n_steps - t0i)

                # ---- z bank accumulate: + I@That_{j-1} [+ I@c] + Ah@z ----
                last = (j == n_steps)
                nc.tensor.matmul(zb[:, 0:W], ident[:], t_jm1[:, 0:W],
                                 start=False, stop=False, skip_group_check=True)
                if j > CLAG:
                    cinj = cql.pop(0)
                    nc.tensor.matmul(zb[:, 0:W], ident[:], cinj[:, 0:W],
                                     start=False, stop=False,
                                     skip_group_check=True)
                for m in range(2):
                    o = zb[:, m * BLOC:(m + 1) * BLOC]
                    nc.tensor.matmul(o, wa[0][m][:], z[:, 0:BLOC],
                                     start=False, stop=False,
                                     skip_group_check=True)
                    nc.tensor.matmul(o, wa[1][m][:], z[:, BLOC:W],
                                     start=False, stop=(m == 1),
                                     skip_group_check=True)

                # ---- delta-hat bank: Cs @ That_{j-1} ----
                db = dbp.tile([128, W], F32, tag="dbk", name="dbk")
                for m in range(2):
                    o = db[:, m * BLOC:(m + 1) * BLOC]
                    nc.tensor.matmul(o, wcs[0][m][:], t_jm1[:, 0:BLOC],
                                     start=True, stop=False)
                    nc.tensor.matmul(o, wcs[1][m][:], t_jm1[:, BLOC:W],
                                     start=False, stop=True)

                # ---- CAST: z_j = zb -> fp16, straight into DMA staging ----
                if s == 0:
                    stg = stgp.tile([128, GRP, W], F16, tag="stg", name="stg")
                zn = stg[:, s, :]
                nc.vector.tensor_copy(zn, zb[:])

                # ---- Yhat_{j+1} bank + That_{j+1} + Q_{j+1} (uses z_j) ----
                t_jp1 = q_jp1 = None
                if j < n_steps:
                    yb = ybp.tile([128, W], F32, tag="yb", name="yb")
                    ybank_mms(yb, zn)
                    t_jp1 = tpool.tile([128, W], F16, tag="tt", name="tt")
                    nc.scalar.activation(t_jp1[:], yb[:], AF.Tanh)
                    q_jp1 = qpool.tile([128, W], F16, tag="qq", name="qq")
                    nc.scalar.activation(q_jp1[:], t_jp1[:], AF.Square)

                # ---- S_{j-1} and c_{j-1} (one iteration late, off-chain) ----
                if 2 <= j <= n_steps - CLAG + 1:
                    ss = spool.tile([128, W], F16, tag="ss", name="ss")
                    nc.vector.tensor_scalar(ss[:], q_jm1[:], -1.0, 1.0,
                                            op0=ALU.mult, op1=ALU.add)
                    cc = cpool.tile([128, W], F16, tag="cc", name="cc")
                    nc.vector.scalar_tensor_tensor(
                        cc[:], db_jm1[:], 1.0, ss[:],
                        op0=ALU.mult, op1=ALU.mult)
                    cql.append(cc)

                if s == blk - 1:
                    nc.sync.dma_start(OUT[:, t0i:t0i + blk, :],
                                      stg[:, 0:blk, :])

                z = zn
                t_jm1 = t_j
                t_j = t_jp1
                q_jm1 = q_j
                q_j = q_jp1
                db_jm1 = db
    nc.compile()
    return nc


def kernel(X0, MA, MW, bA_z, bW_z, by_w):
    global LAST_RESULT
    from concourse.bass_utils import run_bass_kernel_spmd

    X0 = np.asarray(X0, dtype=np.float32)
    MA = np.asarray(MA, dtype=np.float32)
    MW = np.asarray(MW, dtype=np.float32)
    bA_z = np.asarray(bA_z, dtype=np.float32)
    bW_z = np.asarray(bW_z, dtype=np.float32)
    by_w = np.asarray(by_w, dtype=np.float32)

    bA = np.float32(0.5) * np.exp(-bA_z[0, 0] * bA_z[0, 0]) + np.float32(0.5)
    bW = np.float32(0.5) * np.exp(-bW_z[0, 0] * bW_z[0, 0]) + np.float32(0.5)
    I = np.eye(N, dtype=np.float32)
    A = (1 - bA) * (MA + MA.T) + bA * (MA - MA.T) - np.float32(YA) * I
    C = (1 - bA) * (MW + MW.T) + bW * (MW - MW.T) - np.float32(YW) * I

    f16 = lambda x: x.astype(np.float16).astype(np.float32)
    Ah = f16(np.float32(STEP) * A)
    Cs = f16(np.float32(STEP) * C)
    CsD = f16((np.float32(STEP) * C) @ (I + np.float32(STEP) * A))
    byh = f16(by_w)

    WAh = np.ascontiguousarray(Ah.T).astype(np.float16)
    WCSh = np.ascontiguousarray(Cs.T).astype(np.float16)
    WCDh = np.ascontiguousarray(CsD.T).astype(np.float16)
    BPh = np.zeros((2, 128, 128), dtype=np.float16)
    BPh[0, 0, :] = byh[0:128, 0].astype(np.float16)
    BPh[1, 0, :] = byh[128:256, 0].astype(np.float16)
    E0h = np.zeros((128, BLOC), dtype=np.float16)
    E0h[0, :] = 1.0

    in_maps = []
    for i in range(NCORES):
        Xc = X0[i * BLOC:(i + 1) * BLOC, :] / np.float32(STEP)   # [32, 256]
        z0 = Xc.T.astype(np.float16)                              # [256, 32]
        z0f = z0.astype(np.float32)
        T0f = np.tanh(Cs @ z0f + byh).astype(np.float16)          # [256, 32]
        Z0h = np.empty((128, 2 * BLOC), dtype=np.float16)
        T0h = np.empty((128, 2 * BLOC), dtype=np.float16)
        for c in range(2):
            Z0h[:, c * BLOC:(c + 1) * BLOC] = z0[c * 128:(c + 1) * 128, :]
            T0h[:, c * BLOC:(c + 1) * BLOC] = T0f[c * 128:(c + 1) * 128, :]
        in_maps.append({
            "WA": WAh, "WCS": WCSh, "WCD": WCDh, "BP": BPh, "E0": E0h,
            "Z0": Z0h, "T0": T0h,
        })

    nc = _build(NSTEPS)
    res = run_bass_kernel_spmd(nc, in_maps, core_ids=list(range(NCORES)))
    LAST_RESULT = res

    out = np.empty((BS, TMAX, N), dtype=np.float32)
    out[:, 0, :] = X0
    for i in range(NCORES):
        O = np.asarray(res.results[i]["OUT"]).reshape(128, NSTEPS, 2, BLOC)
        # [p, t, c, b] -> [b, t, c, p] -> [32, 511, 256]
        blockX = O.transpose(3, 1, 2, 0).reshape(BLOC, NSTEPS, N)
        out[i * BLOC:(i + 1) * BLOC, 1:, :] = (
            blockX.astype(np.float32) * np.float32(STEP))
    return out


if __name__ == "__main__":
    rng = np.random.default_rng(0)
    inputs = {
        "X0": rng.standard_normal((BS, N), dtype=np.float32),
        "MA": rng.standard_normal((N, N), dtype=np.float32) / 16,
        "MW": rng.standard_normal((N, N), dtype=np.float32) / 16,
        "bA_z": np.full((1, 1), 0.65, dtype=np.float32),
        "bW_z": np.full((1, 1), 0.65, dtype=np.float32),
        "by_w": rng.standard_normal((N, 1), dtype=np.float32) / 100,
    }
    out = kernel(**inputs)
    print("out", out.shape, out.dtype, np.abs(out).max())


# revision 11
# speedup vs baseline: 1.7877x; 1.0270x over previous
"""LipschitzRNN Trainium2 kernel — correction-chain design.

Math (per reference):
    A = (1-bA)(MA+MA.T) + bA(MA-MA.T) - YA*I ; C likewise with bW mix
    X_{t+1} = X_t + STEP*(A@X_t + tanh(C@X_t + by))
    out[b, t, :] = X_t[:, b]

Device strategy (8-way batch data-parallel, 32 cols/core, no collectives):
  State z = X/STEP in fp16 (STEP-scaled weights keep fp16 relative error;
  the tanh term enters unscaled; fp16 carry random-walks to ~9.5e-3 rel,
  budget 2e-2).  The serial per-step chain is the wall-clock limit
  (engines are mostly idle), so the tanh is taken OFF the chain by a
  first-order Taylor correction:

      Yhat_j = by + (Cs D)@z_{j-1}          (one step of slack; D = I+Ah)
      That_j = tanh(Yhat_j),  S_j = 1 - That_j^2
      c_j    = (Cs@That_{j-1}) * S_j        (first-order delta correction)
      z_j    = z_{j-1} + Ah@z_{j-1} + That_{j-1} [+ c_{j-4}, injected late]

  with Ah = STEP*A, Cs = STEP*C (fp16), CsD = f16(STEP*C@(I+STEP*A)).
  The only true serial chain is  z -> {I,Ah}-matmuls -> CAST -> z
  (~700ns); tanh/Square/S/c ride parallel tracks with >=1 step slack, and
  the tiny correction (|c|~0.05 in z units ~ 1e-4 of X) enters 3 steps
  late, which the numpy model shows costs nothing (rel err 9.53e-3).
  Bias is folded into the Yhat bank with padded-128 "bias weight"
  matmuls (row 0 = by chunk) against a constant e0 tile, so a single
  [128,64] tanh instruction needs no per-partition bias and the PE never
  switches tile configs (k=1 matmuls cost ~100ns reconfig stalls).
  Output: z blocks [128, 16 steps, 64] DMA'd raw every 16 steps
  (2KB/descriptor); host transposes [p,t,c*32+b] -> [b,t,n] and scales
  by STEP.  No PE transposes, no staging copies.
"""

import numpy as np

N = 256
BS = 256
TMAX = 512
STEP = 0.01
YA = 0.001
YW = 0.001
NCORES = 8
BLOC = BS // NCORES      # 32 batch cols per core
GRP = 16                 # output steps per DMA block
NSTEPS = TMAX - 1        # 511
CLAG = 4                 # c_{j-CLAG} injected into zb_j

LAST_RESULT = None  # BassKernelResults of the most recent run (for test harness)


def _build(n_steps):
    from concourse import bacc, tile
    import concourse.mybir as mybir
    from concourse.masks import make_identity

    F32 = mybir.dt.float32
    F16 = mybir.dt.float16
    AF = mybir.ActivationFunctionType
    ALU = mybir.AluOpType

    nc = bacc.Bacc("TRN2", target_bir_lowering=False, debug=False,
                   num_devices=NCORES)

    WA = nc.dram_tensor("WA", [N, N], F16, kind="ExternalInput")    # (STEP*A).T
    WCS = nc.dram_tensor("WCS", [N, N], F16, kind="ExternalInput")  # (STEP*C).T
    WCD = nc.dram_tensor("WCD", [N, N], F16, kind="ExternalInput")  # (STEP*C*D).T
    BP = nc.dram_tensor("BP", [2, 128, 128], F16, kind="ExternalInput")
    E0 = nc.dram_tensor("E0", [128, BLOC], F16, kind="ExternalInput")
    Z0 = nc.dram_tensor("Z0", [128, 2 * BLOC], F16, kind="ExternalInput")
    T0 = nc.dram_tensor("T0", [128, 2 * BLOC], F16, kind="ExternalInput")
    OUT = nc.dram_tensor("OUT", [128, n_steps, 2 * BLOC], F16,
                         kind="ExternalOutput")

    W = 2 * BLOC  # 64: working tile width (2 n-chunks x 32 batch)

    with tile.TileContext(nc) as tc:
        with (
            tc.tile_pool(name="consts", bufs=1) as consts,
            tc.tile_pool(name="tpool", bufs=3) as tpool,
            tc.tile_pool(name="qpool", bufs=3) as qpool,
            tc.tile_pool(name="spool", bufs=2) as spool,
            tc.tile_pool(name="cpool", bufs=CLAG + 3) as cpool,
            tc.tile_pool(name="stg", bufs=2) as stgp,
            tc.tile_pool(name="zb", bufs=1, space="PSUM") as zbp,
            tc.tile_pool(name="yb", bufs=3, space="PSUM") as ybp,
            tc.tile_pool(name="db", bufs=3, space="PSUM") as dbp,
        ):
            # ---- constants / initial state ----
            wa = [[consts.tile([128, 128], F16, tag=f"wa{k}{m}", name=f"wa{k}{m}")
                   for m in range(2)] for k in range(2)]
            wcs = [[consts.tile([128, 128], F16, tag=f"wcs{k}{m}", name=f"wcs{k}{m}")
                    for m in range(2)] for k in range(2)]
            wcd = [[consts.tile([128, 128], F16, tag=f"wcd{k}{m}", name=f"wcd{k}{m}")
                    for m in range(2)] for k in range(2)]
            for k in range(2):
                for m in range(2):
                    sl = (slice(128 * k, 128 * (k + 1)), slice(128 * m, 128 * (m + 1)))
                    nc.sync.dma_start(wa[k][m][:], WA[sl[0], sl[1]])
                    nc.sync.dma_start(wcs[k][m][:], WCS[sl[0], sl[1]])
                    nc.sync.dma_start(wcd[k][m][:], WCD[sl[0], sl[1]])
            bp = [consts.tile([128, 128], F16, tag=f"bp{m}", name=f"bp{m}")
                  for m in range(2)]
            nc.sync.dma_start(bp[0][:], BP[0])
            nc.sync.dma_start(bp[1][:], BP[1])
            e0 = consts.tile([128, BLOC], F16, tag="e0", name="e0")
            nc.sync.dma_start(e0[:], E0[:, :])
            ident_f32 = consts.tile([128, 128], F32, tag="idf", name="idf")
            make_identity(nc, ident_f32[:])
            ident = consts.tile([128, 128], F16, tag="ident", name="ident")
            nc.vector.tensor_copy(ident[:], ident_f32[:])
            z0t = consts.tile([128, W], F16, tag="z0t", name="z0t")
            nc.sync.dma_start(z0t[:], Z0[:, :])
            t0t = consts.tile([128, W], F16, tag="t0t", name="t0t")
            nc.sync.dma_start(t0t[:], T0[:, :])

            def ybank_mms(yb, zsrc):
                # Yhat bank: bias (padded weights vs e0) + CsD @ z
                nc.tensor.matmul(yb[:, 0:BLOC], bp[0][:], e0[:],
                                 start=True, stop=False)
                nc.tensor.matmul(yb[:, BLOC:W], bp[1][:], e0[:],
                                 start=True, stop=False)
                for m in range(2):
                    o = yb[:, m * BLOC:(m + 1) * BLOC]
                    nc.tensor.matmul(o, wcd[0][m][:], zsrc[:, 0:BLOC],
                                     start=False, stop=False)
                    nc.tensor.matmul(o, wcd[1][m][:], zsrc[:, BLOC:W],
                                     start=False, stop=True)

            # ---- persistent f32 z-bank: init M = I@z0, then accumulate
            # Ah@z + That + c onto it forever (exact f32 carry) ----
            zb = zbp.tile([128, W], F32, tag="zbk", name="zbk")
            nc.tensor.matmul(zb[:, 0:W], ident[:], z0t[:, 0:W],
                             start=True, stop=False, skip_group_check=True)

            t_jm1 = t0t             # That_{j-1} (That_0 := T0)
            q_jm1 = None            # Q_{j-1}
            db_jm1 = None           # delta-hat bank from prev iter
            cql = []                # pending c tiles (FIFO)
            z = z0t[:]
            stg = None

            for j in range(1, n_steps + 1):
                s = (j - 1) % GRP
                t0i = j - 1 - s
                blk = min(GRP, n_steps - t0i)

                # ---- z bank accumulate: + Ah@z [+ I@c] + I@That_{j-1} ----
                # (I@That last: That_{j-1} is the latest-arriving input)
                for m in range(2):
                    o = zb[:, m * BLOC:(m + 1) * BLOC]
                    nc.tensor.matmul(o, wa[0][m][:], z[:, 0:BLOC],
                                     start=False, stop=False,
                                     skip_group_check=True)
                    nc.tensor.matmul(o, wa[1][m][:], z[:, BLOC:W],
                                     start=False, stop=False,
                                     skip_group_check=True)
                if j > CLAG:
                    cinj = cql.pop(0)
                    nc.tensor.matmul(zb[:, 0:W], ident[:], cinj[:, 0:W],
                                     start=False, stop=False,
                                     skip_group_check=True)
                nc.tensor.matmul(zb[:, 0:W], ident[:], t_jm1[:, 0:W],
                                 start=False, stop=True, skip_group_check=True)

                # ---- Yhat_j bank (bias + CsD@z_{j-1}) + That_j + Q_j ----
                yb = ybp.tile([128, W], F32, tag="yb", name="yb")
                ybank_mms(yb, z)
                t_j = tpool.tile([128, W], F16, tag="tt", name="tt")
                nc.scalar.activation(t_j[:], yb[:], AF.Tanh)
                q_j = qpool.tile([128, W], F16, tag="qq", name="qq")
                nc.scalar.activation(q_j[:], t_j[:], AF.Square)

                # ---- delta-hat bank: Cs @ That_{j-1} ----
                db = dbp.tile([128, W], F32, tag="dbk", name="dbk")
                for m in range(2):
                    o = db[:, m * BLOC:(m + 1) * BLOC]
                    nc.tensor.matmul(o, wcs[0][m][:], t_jm1[:, 0:BLOC],
                                     start=True, stop=False)
                    nc.tensor.matmul(o, wcs[1][m][:], t_jm1[:, BLOC:W],
                                     start=False, stop=True)

                # ---- CAST: z_j = zb -> fp16, straight into DMA staging ----
                if s == 0:
                    stg = stgp.tile([128, GRP, W], F16, tag="stg", name="stg")
                zn = stg[:, s, :]
                nc.vector.tensor_copy(zn, zb[:])

                # ---- S_{j-1} and c_{j-1} (one iteration late, off-chain) ----
                if 2 <= j <= n_steps - CLAG + 1:
                    ss = spool.tile([128, W], F16, tag="ss", name="ss")
                    nc.vector.tensor_scalar(ss[:], q_jm1[:], -1.0, 1.0,
                                            op0=ALU.mult, op1=ALU.add)
                    cc = cpool.tile([128, W], F16, tag="cc", name="cc")
                    nc.vector.scalar_tensor_tensor(
                        cc[:], db_jm1[:], 1.0, ss[:],
                        op0=ALU.mult, op1=ALU.mult)
                    cql.append(cc)

                if s == blk - 1:
                    nc.sync.dma_start(OUT[:, t0i:t0i + blk, :],
                                      stg[:, 0:blk, :])

                z = zn
                t_jm1 = t_j
                q_jm1 = q_j
                db_jm1 = db
    nc.compile()
    return nc


def kernel(X0, MA, MW, bA_z, bW_z, by_w):
    global LAST_RESULT
    from concourse.bass_utils import run_bass_kernel_spmd

    X0 = np.asarray(X0, dtype=np.float32)
    MA = np.asarray(MA, dtype=np.float32)
    MW = np.asarray(MW, dtype=np.float32)
    bA_z = np.asarray(bA_z, dtype=np.float32)
    bW_z = np.asarray(bW_z, dtype=np.float32)
    by_w = np.asarray(by_w, dtype=np.float32)

    bA = np.float32(0.5) * np.exp(-bA_z[0, 0] * bA_z[0, 0]) + np.float32(0.5)
    bW = np.float32(0.5) * np.exp(-bW_z[0, 0] * bW_z[0, 0]) + np.float32(0.5)
    I = np.eye(N, dtype=np.float32)
    A = (1 - bA) * (MA + MA.T) + bA * (MA - MA.T) - np.float32(YA) * I
    C = (1 - bA) * (MW + MW.T) + bW * (MW - MW.T) - np.float32(YW) * I

    f16 = lambda x: x.astype(np.float16).astype(np.float32)
    Ah = f16(np.float32(STEP) * A)
    Cs = f16(np.float32(STEP) * C)
    CsD = f16((np.float32(STEP) * C) @ (I + np.float32(STEP) * A))
    byh = f16(by_w)

    WAh = np.ascontiguousarray(Ah.T).astype(np.float16)
    WCSh = np.ascontiguousarray(Cs.T).astype(np.float16)
    WCDh = np.ascontiguousarray(CsD.T).astype(np.float16)
    BPh = np.zeros((2, 128, 128), dtype=np.float16)
    BPh[0, 0, :] = byh[0:128, 0].astype(np.float16)
    BPh[1, 0, :] = byh[128:256, 0].astype(np.float16)
    E0h = np.zeros((128, BLOC), dtype=np.float16)
    E0h[0, :] = 1.0

    in_maps = []
    for i in range(NCORES):
        Xc = X0[i * BLOC:(i + 1) * BLOC, :] / np.float32(STEP)   # [32, 256]
        z0 = Xc.T.astype(np.float16)                              # [256, 32]
        z0f = z0.astype(np.float32)
        T0f = np.tanh(Cs @ z0f + byh).astype(np.float16)          # [256, 32]
        Z0h = np.empty((128, 2 * BLOC), dtype=np.float16)
        T0h = np.empty((128, 2 * BLOC), dtype=np.float16)
        for c in range(2):
            Z0h[:, c * BLOC:(c + 1) * BLOC] = z0[c * 128:(c + 1) * 128, :]
            T0h[:, c * BLOC:(c + 1) * BLOC] = T0f[c * 128:(c + 1) * 128, :]
        in_maps.append({
            "WA": WAh, "WCS": WCSh, "WCD": WCDh, "BP": BPh, "E0": E0h,
            "Z0": Z0h, "T0": T0h,
        })

    nc = _build(NSTEPS)
    res = run_bass_kernel_spmd(nc, in_maps, core_ids=list(range(NCORES)))
    LAST_RESULT = res

    out = np.empty((BS, TMAX, N), dtype=np.float32)
    out[:, 0, :] = X0
    for i in range(NCORES):
        O = np.asarray(res.results[i]["OUT"]).reshape(128, NSTEPS, 2, BLOC)
        # [p, t, c, b] -> [b, t, c, p] -> [32, 511, 256]
        blockX = O.transpose(3, 1, 2, 0).reshape(BLOC, NSTEPS, N)
        out[i * BLOC:(i + 1) * BLOC, 1:, :] = (
            blockX.astype(np.float32) * np.float32(STEP))
    return out


if __name__ == "__main__":
    rng = np.random.default_rng(0)
    inputs = {
        "X0": rng.standard_normal((BS, N), dtype=np.float32),
        "MA": rng.standard_normal((N, N), dtype=np.float32) / 16,
        "MW": rng.standard_normal((N, N), dtype=np.float32) / 16,
        "bA_z": np.full((1, 1), 0.65, dtype=np.float32),
        "bW_z": np.full((1, 1), 0.65, dtype=np.float32),
        "by_w": rng.standard_normal((N, 1), dtype=np.float32) / 100,
    }
    out = kernel(**inputs)
    print("out", out.shape, out.dtype, np.abs(out).max())


# revision 12
# speedup vs baseline: 2.0674x; 1.1564x over previous
"""LipschitzRNN Trainium2 kernel — correction-chain design.

Math (per reference):
    A = (1-bA)(MA+MA.T) + bA(MA-MA.T) - YA*I ; C likewise with bW mix
    X_{t+1} = X_t + STEP*(A@X_t + tanh(C@X_t + by))
    out[b, t, :] = X_t[:, b]

Device strategy (8-way batch data-parallel, 32 cols/core, no collectives):
  State z = X/STEP in fp16 (STEP-scaled weights keep fp16 relative error;
  the tanh term enters unscaled; fp16 carry random-walks to ~9.5e-3 rel,
  budget 2e-2).  The serial per-step chain is the wall-clock limit
  (engines are mostly idle), so the tanh is taken OFF the chain by a
  first-order Taylor correction:

      Yhat_j = by + (Cs D)@z_{j-1}          (one step of slack; D = I+Ah)
      That_j = tanh(Yhat_j),  S_j = 1 - That_j^2
      c_j    = (Cs@That_{j-1}) * S_j        (first-order delta correction)
      z_j    = z_{j-1} + Ah@z_{j-1} + That_{j-1} [+ c_{j-4}, injected late]

  with Ah = STEP*A, Cs = STEP*C (fp16), CsD = f16(STEP*C@(I+STEP*A)).
  The only true serial chain is  z -> {I,Ah}-matmuls -> CAST -> z
  (~700ns); tanh/Square/S/c ride parallel tracks with >=1 step slack, and
  the tiny correction (|c|~0.05 in z units ~ 1e-4 of X) enters 3 steps
  late, which the numpy model shows costs nothing (rel err 9.53e-3).
  Bias is folded into the Yhat bank with padded-128 "bias weight"
  matmuls (row 0 = by chunk) against a constant e0 tile, so a single
  [128,64] tanh instruction needs no per-partition bias and the PE never
  switches tile configs (k=1 matmuls cost ~100ns reconfig stalls).
  Output: z blocks [128, 16 steps, 64] DMA'd raw every 16 steps
  (2KB/descriptor); host transposes [p,t,c*32+b] -> [b,t,n] and scales
  by STEP.  No PE transposes, no staging copies.
"""

import numpy as np

N = 256
BS = 256
TMAX = 512
STEP = 0.01
YA = 0.001
YW = 0.001
NCORES = 8
BLOC = BS // NCORES      # 32 batch cols per core
GRP = 16                 # output steps per DMA block
NSTEPS = TMAX - 1        # 511
CLAG = 4                 # c_{j-CLAG} injected into zb_j

LAST_RESULT = None  # BassKernelResults of the most recent run (for test harness)


def _build(n_steps):
    from concourse import bacc, tile
    import concourse.mybir as mybir
    from concourse.masks import make_identity

    F32 = mybir.dt.float32
    F16 = mybir.dt.float16
    AF = mybir.ActivationFunctionType
    ALU = mybir.AluOpType

    nc = bacc.Bacc("TRN2", target_bir_lowering=False, debug=False,
                   num_devices=NCORES)

    WA = nc.dram_tensor("WA", [N, N], F16, kind="ExternalInput")    # (STEP*A).T
    WCS = nc.dram_tensor("WCS", [N, N], F16, kind="ExternalInput")  # (STEP*C).T
    WCD = nc.dram_tensor("WCD", [N, N], F16, kind="ExternalInput")  # (STEP*C*D).T
    WCD2 = nc.dram_tensor("WCD2", [N, N], F16, kind="ExternalInput")  # (STEP*C*D^2).T
    BP = nc.dram_tensor("BP", [2, 128, 128], F16, kind="ExternalInput")
    E0 = nc.dram_tensor("E0", [128, BLOC], F16, kind="ExternalInput")
    Z0 = nc.dram_tensor("Z0", [128, 2 * BLOC], F16, kind="ExternalInput")
    T0 = nc.dram_tensor("T0", [128, 2 * BLOC], F16, kind="ExternalInput")
    OUT = nc.dram_tensor("OUT", [128, n_steps, 2 * BLOC], F16,
                         kind="ExternalOutput")

    W = 2 * BLOC  # 64: working tile width (2 n-chunks x 32 batch)

    with tile.TileContext(nc) as tc:
        with (
            tc.tile_pool(name="consts", bufs=1) as consts,
            tc.tile_pool(name="tpool", bufs=4) as tpool,
            tc.tile_pool(name="qpool", bufs=3) as qpool,
            tc.tile_pool(name="spool", bufs=2) as spool,
            tc.tile_pool(name="cpool", bufs=CLAG + 3) as cpool,
            tc.tile_pool(name="stg", bufs=2) as stgp,
            tc.tile_pool(name="zb", bufs=1, space="PSUM") as zbp,
            tc.tile_pool(name="yb", bufs=3, space="PSUM") as ybp,
            tc.tile_pool(name="db", bufs=3, space="PSUM") as dbp,
        ):
            # ---- constants / initial state ----
            wa = [[consts.tile([128, 128], F16, tag=f"wa{k}{m}", name=f"wa{k}{m}")
                   for m in range(2)] for k in range(2)]
            wcs = [[consts.tile([128, 128], F16, tag=f"wcs{k}{m}", name=f"wcs{k}{m}")
                    for m in range(2)] for k in range(2)]
            wcd = [[consts.tile([128, 128], F16, tag=f"wcd{k}{m}", name=f"wcd{k}{m}")
                    for m in range(2)] for k in range(2)]
            wcd2 = [[consts.tile([128, 128], F16, tag=f"wcd2{k}{m}", name=f"wcd2{k}{m}")
                     for m in range(2)] for k in range(2)]
            for k in range(2):
                for m in range(2):
                    sl = (slice(128 * k, 128 * (k + 1)), slice(128 * m, 128 * (m + 1)))
                    nc.sync.dma_start(wa[k][m][:], WA[sl[0], sl[1]])
                    nc.sync.dma_start(wcs[k][m][:], WCS[sl[0], sl[1]])
                    nc.sync.dma_start(wcd[k][m][:], WCD[sl[0], sl[1]])
                    nc.sync.dma_start(wcd2[k][m][:], WCD2[sl[0], sl[1]])
            bp = [consts.tile([128, 128], F16, tag=f"bp{m}", name=f"bp{m}")
                  for m in range(2)]
            nc.sync.dma_start(bp[0][:], BP[0])
            nc.sync.dma_start(bp[1][:], BP[1])
            e0 = consts.tile([128, BLOC], F16, tag="e0", name="e0")
            nc.sync.dma_start(e0[:], E0[:, :])
            ident_f32 = consts.tile([128, 128], F32, tag="idf", name="idf")
            make_identity(nc, ident_f32[:])
            ident = consts.tile([128, 128], F16, tag="ident", name="ident")
            nc.vector.tensor_copy(ident[:], ident_f32[:])
            z0t = consts.tile([128, W], F16, tag="z0t", name="z0t")
            nc.sync.dma_start(z0t[:], Z0[:, :])
            t0t = consts.tile([128, W], F16, tag="t0t", name="t0t")
            nc.sync.dma_start(t0t[:], T0[:, :])

            def apply_mat(bank, wt, src_, start, stop):
                for m in range(2):
                    o = bank[:, m * BLOC:(m + 1) * BLOC]
                    nc.tensor.matmul(o, wt[0][m][:], src_[:, 0:BLOC],
                                     start=start, stop=False)
                    nc.tensor.matmul(o, wt[1][m][:], src_[:, BLOC:W],
                                     start=False, stop=stop)

            def bias_mms(yb):
                nc.tensor.matmul(yb[:, 0:BLOC], bp[0][:], e0[:],
                                 start=True, stop=False)
                nc.tensor.matmul(yb[:, BLOC:W], bp[1][:], e0[:],
                                 start=True, stop=False)

            # ---- persistent f32 z-bank: init M = I@z0, then accumulate
            # Ah@z + That + c onto it forever (exact f32 carry) ----
            zb = zbp.tile([128, W], F32, tag="zbk", name="zbk")
            nc.tensor.matmul(zb[:, 0:W], ident[:], z0t[:, 0:W],
                             start=True, stop=False, skip_group_check=True)

            t_jm1 = t0t             # That_{j-1} (That_0 := T0)
            t_m2 = None             # That_{j-2}
            q_jm1 = None            # Q_{j-1}
            db_jm1 = None           # delta-hat bank from prev iter
            cql = []                # pending c tiles (FIFO)
            z = z0t[:]
            z_m2 = None             # z_{j-2}
            stg = None

            for j in range(1, n_steps + 1):
                s = (j - 1) % GRP
                t0i = j - 1 - s
                blk = min(GRP, n_steps - t0i)

                # ---- z bank accumulate: + Ah@z [+ I@c] + I@That_{j-1} ----
                # (I@That last: That_{j-1} is the latest-arriving input)
                for m in range(2):
                    o = zb[:, m * BLOC:(m + 1) * BLOC]
                    nc.tensor.matmul(o, wa[0][m][:], z[:, 0:BLOC],
                                     start=False, stop=False,
                                     skip_group_check=True)
                    nc.tensor.matmul(o, wa[1][m][:], z[:, BLOC:W],
                                     start=False, stop=False,
                                     skip_group_check=True)
                if j > CLAG:
                    cinj = cql.pop(0)
                    nc.tensor.matmul(zb[:, 0:W], ident[:], cinj[:, 0:W],
                                     start=False, stop=False,
                                     skip_group_check=True)
                nc.tensor.matmul(zb[:, 0:W], ident[:], t_jm1[:, 0:W],
                                 start=False, stop=True, skip_group_check=True)

                # ---- Yhat_j bank + That_j + Q_j ----
                # j>=2: by + CsD2@z_{j-2} + CsD@That_{j-2}   (2-step slack)
                yb = ybp.tile([128, W], F32, tag="yb", name="yb")
                bias_mms(yb)
                if j == 1:
                    apply_mat(yb, wcd, z, start=False, stop=True)
                else:
                    apply_mat(yb, wcd2, z_m2, start=False, stop=False)
                    apply_mat(yb, wcd, t_m2, start=False, stop=True)
                t_j = tpool.tile([128, W], F16, tag="tt", name="tt")
                nc.scalar.activation(t_j[:], yb[:], AF.Tanh)
                q_j = qpool.tile([128, W], F16, tag="qq", name="qq")
                nc.scalar.activation(q_j[:], t_j[:], AF.Square)

                # ---- delta-hat bank: Cs @ That_{j-1} ----
                db = dbp.tile([128, W], F32, tag="dbk", name="dbk")
                for m in range(2):
                    o = db[:, m * BLOC:(m + 1) * BLOC]
                    nc.tensor.matmul(o, wcs[0][m][:], t_jm1[:, 0:BLOC],
                                     start=True, stop=False)
                    nc.tensor.matmul(o, wcs[1][m][:], t_jm1[:, BLOC:W],
                                     start=False, stop=True)

                # ---- CAST: z_j = zb -> fp16, straight into DMA staging ----
                if s == 0:
                    stg = stgp.tile([128, GRP, W], F16, tag="stg", name="stg")
                zn = stg[:, s, :]
                nc.vector.tensor_copy(zn, zb[:])

                # ---- S_{j-1} and c_{j-1} (one iteration late, off-chain) ----
                if 2 <= j <= n_steps - CLAG + 1:
                    ss = spool.tile([128, W], F16, tag="ss", name="ss")
                    nc.vector.tensor_scalar(ss[:], q_jm1[:], -1.0, 1.0,
                                            op0=ALU.mult, op1=ALU.add)
                    cc = cpool.tile([128, W], F16, tag="cc", name="cc")
                    nc.vector.scalar_tensor_tensor(
                        cc[:], db_jm1[:], 1.0, ss[:],
                        op0=ALU.mult, op1=ALU.mult)
                    cql.append(cc)

                if s == blk - 1:
                    nc.sync.dma_start(OUT[:, t0i:t0i + blk, :],
                                      stg[:, 0:blk, :])

                z_m2 = z
                z = zn
                t_m2 = t_jm1
                t_jm1 = t_j
                q_jm1 = q_j
                db_jm1 = db
    nc.compile()
    return nc


def kernel(X0, MA, MW, bA_z, bW_z, by_w):
    global LAST_RESULT
    from concourse.bass_utils import run_bass_kernel_spmd

    X0 = np.asarray(X0, dtype=np.float32)
    MA = np.asarray(MA, dtype=np.float32)
    MW = np.asarray(MW, dtype=np.float32)
    bA_z = np.asarray(bA_z, dtype=np.float32)
    bW_z = np.asarray(bW_z, dtype=np.float32)
    by_w = np.asarray(by_w, dtype=np.float32)

    bA = np.float32(0.5) * np.exp(-bA_z[0, 0] * bA_z[0, 0]) + np.float32(0.5)
    bW = np.float32(0.5) * np.exp(-bW_z[0, 0] * bW_z[0, 0]) + np.float32(0.5)
    I = np.eye(N, dtype=np.float32)
    A = (1 - bA) * (MA + MA.T) + bA * (MA - MA.T) - np.float32(YA) * I
    C = (1 - bA) * (MW + MW.T) + bW * (MW - MW.T) - np.float32(YW) * I

    f16 = lambda x: x.astype(np.float16).astype(np.float32)
    Ah = f16(np.float32(STEP) * A)
    Cs = f16(np.float32(STEP) * C)
    Dm = I + np.float32(STEP) * A
    CsD = f16((np.float32(STEP) * C) @ Dm)
    CsD2 = f16((np.float32(STEP) * C) @ Dm @ Dm)
    byh = f16(by_w)

    WAh = np.ascontiguousarray(Ah.T).astype(np.float16)
    WCSh = np.ascontiguousarray(Cs.T).astype(np.float16)
    WCDh = np.ascontiguousarray(CsD.T).astype(np.float16)
    WCD2h = np.ascontiguousarray(CsD2.T).astype(np.float16)
    BPh = np.zeros((2, 128, 128), dtype=np.float16)
    BPh[0, 0, :] = byh[0:128, 0].astype(np.float16)
    BPh[1, 0, :] = byh[128:256, 0].astype(np.float16)
    E0h = np.zeros((128, BLOC), dtype=np.float16)
    E0h[0, :] = 1.0

    in_maps = []
    for i in range(NCORES):
        Xc = X0[i * BLOC:(i + 1) * BLOC, :] / np.float32(STEP)   # [32, 256]
        z0 = Xc.T.astype(np.float16)                              # [256, 32]
        z0f = z0.astype(np.float32)
        T0f = np.tanh(Cs @ z0f + byh).astype(np.float16)          # [256, 32]
        Z0h = np.empty((128, 2 * BLOC), dtype=np.float16)
        T0h = np.empty((128, 2 * BLOC), dtype=np.float16)
        for c in range(2):
            Z0h[:, c * BLOC:(c + 1) * BLOC] = z0[c * 128:(c + 1) * 128, :]
            T0h[:, c * BLOC:(c + 1) * BLOC] = T0f[c * 128:(c + 1) * 128, :]
        in_maps.append({
            "WA": WAh, "WCS": WCSh, "WCD": WCDh, "WCD2": WCD2h, "BP": BPh, "E0": E0h,
            "Z0": Z0h, "T0": T0h,
        })

    nc = _build(NSTEPS)
    res = run_bass_kernel_spmd(nc, in_maps, core_ids=list(range(NCORES)))
    LAST_RESULT = res

    out = np.empty((BS, TMAX, N), dtype=np.float32)
    out[:, 0, :] = X0
    for i in range(NCORES):
        O = np.asarray(res.results[i]["OUT"]).reshape(128, NSTEPS, 2, BLOC)
        # [p, t, c, b] -> [b, t, c, p] -> [32, 511, 256]
        blockX = O.transpose(3, 1, 2, 0).reshape(BLOC, NSTEPS, N)
        out[i * BLOC:(i + 1) * BLOC, 1:, :] = (
            blockX.astype(np.float32) * np.float32(STEP))
    return out


if __name__ == "__main__":
    rng = np.random.default_rng(0)
    inputs = {
        "X0": rng.standard_normal((BS, N), dtype=np.float32),
        "MA": rng.standard_normal((N, N), dtype=np.float32) / 16,
        "MW": rng.standard_normal((N, N), dtype=np.float32) / 16,
        "bA_z": np.full((1, 1), 0.65, dtype=np.float32),
        "bW_z": np.full((1, 1), 0.65, dtype=np.float32),
        "by_w": rng.standard_normal((N, 1), dtype=np.float32) / 100,
    }
    out = kernel(**inputs)
    print("out", out.shape, out.dtype, np.abs(out).max())


# revision 13
# speedup vs baseline: 2.1261x; 1.0284x over previous
"""LipschitzRNN Trainium2 kernel — correction-chain design.

Math (per reference):
    A = (1-bA)(MA+MA.T) + bA(MA-MA.T) - YA*I ; C likewise with bW mix
    X_{t+1} = X_t + STEP*(A@X_t + tanh(C@X_t + by))
    out[b, t, :] = X_t[:, b]

Device strategy (8-way batch data-parallel, 32 cols/core, no collectives):
  State z = X/STEP in fp16 (STEP-scaled weights keep fp16 relative error;
  the tanh term enters unscaled; fp16 carry random-walks to ~9.5e-3 rel,
  budget 2e-2).  The serial per-step chain is the wall-clock limit
  (engines are mostly idle), so the tanh is taken OFF the chain by a
  first-order Taylor correction:

      Yhat_j = by + (Cs D)@z_{j-1}          (one step of slack; D = I+Ah)
      That_j = tanh(Yhat_j),  S_j = 1 - That_j^2
      c_j    = (Cs@That_{j-1}) * S_j        (first-order delta correction)
      z_j    = z_{j-1} + Ah@z_{j-1} + That_{j-1} [+ c_{j-4}, injected late]

  with Ah = STEP*A, Cs = STEP*C (fp16), CsD = f16(STEP*C@(I+STEP*A)).
  The only true serial chain is  z -> {I,Ah}-matmuls -> CAST -> z
  (~700ns); tanh/Square/S/c ride parallel tracks with >=1 step slack, and
  the tiny correction (|c|~0.05 in z units ~ 1e-4 of X) enters 3 steps
  late, which the numpy model shows costs nothing (rel err 9.53e-3).
  Bias is folded into the Yhat bank with padded-128 "bias weight"
  matmuls (row 0 = by chunk) against a constant e0 tile, so a single
  [128,64] tanh instruction needs no per-partition bias and the PE never
  switches tile configs (k=1 matmuls cost ~100ns reconfig stalls).
  Output: z blocks [128, 16 steps, 64] DMA'd raw every 16 steps
  (2KB/descriptor); host transposes [p,t,c*32+b] -> [b,t,n] and scales
  by STEP.  No PE transposes, no staging copies.
"""

import numpy as np

N = 256
BS = 256
TMAX = 512
STEP = 0.01
YA = 0.001
YW = 0.001
NCORES = 8
BLOC = BS // NCORES      # 32 batch cols per core
GRP = 16                 # output steps per DMA block
NSTEPS = TMAX - 1        # 511
CLAG = 4                 # c_{j-CLAG} injected into zb_j

LAST_RESULT = None  # BassKernelResults of the most recent run (for test harness)


def _build(n_steps):
    from concourse import bacc, tile
    import concourse.mybir as mybir
    from concourse.masks import make_identity

    F32 = mybir.dt.float32
    F16 = mybir.dt.float16
    AF = mybir.ActivationFunctionType
    ALU = mybir.AluOpType

    nc = bacc.Bacc("TRN2", target_bir_lowering=False, debug=False,
                   num_devices=NCORES)

    # all weights/constants/init state packed into one tensor: a single
    # input DMA instead of 21 serialized ~650ns ones (saves ~12us startup)
    NPACK = 19 * 128 + BLOC + 4 * BLOC
    WPACK = nc.dram_tensor("WPACK", [128, NPACK], F16, kind="ExternalInput")
    OUT = nc.dram_tensor("OUT", [128, n_steps, 2 * BLOC], F16,
                         kind="ExternalOutput")

    W = 2 * BLOC  # 64: working tile width (2 n-chunks x 32 batch)

    with tile.TileContext(nc) as tc:
        with (
            tc.tile_pool(name="consts", bufs=1) as consts,
            tc.tile_pool(name="tpool", bufs=4) as tpool,
            tc.tile_pool(name="qpool", bufs=3) as qpool,
            tc.tile_pool(name="spool", bufs=2) as spool,
            tc.tile_pool(name="cpool", bufs=CLAG + 3) as cpool,
            tc.tile_pool(name="stg", bufs=2) as stgp,
            tc.tile_pool(name="zb", bufs=1, space="PSUM") as zbp,
            tc.tile_pool(name="yb", bufs=3, space="PSUM") as ybp,
            tc.tile_pool(name="db", bufs=3, space="PSUM") as dbp,
        ):
            # ---- constants / initial state (one packed DMA) ----
            wpk = consts.tile([128, NPACK], F16, tag="wpk", name="wpk")
            nc.sync.dma_start(wpk[:], WPACK[:, :])

            def wsl(i):
                return wpk[:, i * 128:(i + 1) * 128]
            wa = [[wsl(0 + 2 * k + m) for m in range(2)] for k in range(2)]
            wcs = [[wsl(4 + 2 * k + m) for m in range(2)] for k in range(2)]
            wcd = [[wsl(8 + 2 * k + m) for m in range(2)] for k in range(2)]
            wcd2 = [[wsl(12 + 2 * k + m) for m in range(2)] for k in range(2)]
            bp = [wsl(16), wsl(17)]
            ident = wsl(18)
            e0 = wpk[:, 2432:2432 + BLOC]
            z0t = wpk[:, 2464:2464 + 2 * BLOC]
            t0t = wpk[:, 2528:2528 + 2 * BLOC]

            def apply_mat(bank, wt, src_, start, stop):
                for m in range(2):
                    o = bank[:, m * BLOC:(m + 1) * BLOC]
                    nc.tensor.matmul(o, wt[0][m], src_[:, 0:BLOC],
                                     start=start, stop=False)
                    nc.tensor.matmul(o, wt[1][m], src_[:, BLOC:W],
                                     start=False, stop=stop)

            def bias_mms(yb):
                nc.tensor.matmul(yb[:, 0:BLOC], bp[0], e0,
                                 start=True, stop=False)
                nc.tensor.matmul(yb[:, BLOC:W], bp[1], e0,
                                 start=True, stop=False)

            # ---- persistent f32 z-bank: init M = I@z0, then accumulate
            # Ah@z + That + c onto it forever (exact f32 carry) ----
            zb = zbp.tile([128, W], F32, tag="zbk", name="zbk")
            nc.tensor.matmul(zb[:, 0:W], ident, z0t[:, 0:W],
                             start=True, stop=False, skip_group_check=True)

            t_jm1 = t0t
            t_m2 = None             # That_{j-2}
            q_jm1 = None            # Q_{j-1}
            db_jm1 = None           # delta-hat bank from prev iter
            cql = []                # pending c tiles (FIFO)
            z = z0t
            z_m2 = None             # z_{j-2}
            stg = None

            for j in range(1, n_steps + 1):
                s = (j - 1) % GRP
                t0i = j - 1 - s
                blk = min(GRP, n_steps - t0i)

                # ---- z bank accumulate: + Ah@z [+ I@c] + I@That_{j-1} ----
                # (I@That last: That_{j-1} is the latest-arriving input)
                for m in range(2):
                    o = zb[:, m * BLOC:(m + 1) * BLOC]
                    nc.tensor.matmul(o, wa[0][m], z[:, 0:BLOC],
                                     start=False, stop=False,
                                     skip_group_check=True)
                    nc.tensor.matmul(o, wa[1][m], z[:, BLOC:W],
                                     start=False, stop=False,
                                     skip_group_check=True)
                if j > CLAG:
                    cinj = cql.pop(0)
                    nc.tensor.matmul(zb[:, 0:W], ident, cinj[:, 0:W],
                                     start=False, stop=False,
                                     skip_group_check=True)
                nc.tensor.matmul(zb[:, 0:W], ident, t_jm1[:, 0:W],
                                 start=False, stop=True, skip_group_check=True)

                # ---- Yhat_j bank + That_j + Q_j ----
                # j>=2: by + CsD2@z_{j-2} + CsD@That_{j-2}   (2-step slack)
                t_j = q_j = None
                if j < n_steps:
                    yb = ybp.tile([128, W], F32, tag="yb", name="yb")
                    bias_mms(yb)
                    if j == 1:
                        apply_mat(yb, wcd, z, start=False, stop=True)
                    else:
                        apply_mat(yb, wcd2, z_m2, start=False, stop=False)
                        apply_mat(yb, wcd, t_m2, start=False, stop=True)
                    t_j = tpool.tile([128, W], F16, tag="tt", name="tt")
                    nc.scalar.activation(t_j[:], yb[:], AF.Tanh)
                    q_j = qpool.tile([128, W], F16, tag="qq", name="qq")
                    nc.scalar.activation(q_j[:], t_j[:], AF.Square)

                # ---- delta-hat bank: Cs @ That_{j-1} ----
                db = dbp.tile([128, W], F32, tag="dbk", name="dbk")
                for m in range(2):
                    o = db[:, m * BLOC:(m + 1) * BLOC]
                    nc.tensor.matmul(o, wcs[0][m], t_jm1[:, 0:BLOC],
                                     start=True, stop=False)
                    nc.tensor.matmul(o, wcs[1][m], t_jm1[:, BLOC:W],
                                     start=False, stop=True)

                # ---- CAST: z_j = zb -> fp16, straight into DMA staging ----
                if s == 0:
                    stg = stgp.tile([128, GRP, W], F16, tag="stg", name="stg")
                zn = stg[:, s, :]
                nc.vector.tensor_copy(zn, zb[:])

                # ---- S_{j-1} and c_{j-1} (one iteration late, off-chain) ----
                if 2 <= j <= n_steps - CLAG + 1:
                    ss = spool.tile([128, W], F16, tag="ss", name="ss")
                    nc.vector.tensor_scalar(ss[:], q_jm1[:], -1.0, 1.0,
                                            op0=ALU.mult, op1=ALU.add)
                    cc = cpool.tile([128, W], F16, tag="cc", name="cc")
                    nc.vector.scalar_tensor_tensor(
                        cc[:], db_jm1[:], 1.0, ss[:],
                        op0=ALU.mult, op1=ALU.mult)
                    cql.append(cc)

                if s == blk - 1:
                    nc.sync.dma_start(OUT[:, t0i:t0i + blk, :],
                                      stg[:, 0:blk, :])

                z_m2 = z
                z = zn
                t_m2 = t_jm1
                t_jm1 = t_j
                q_jm1 = q_j
                db_jm1 = db
    nc.compile()
    return nc


def kernel(X0, MA, MW, bA_z, bW_z, by_w):
    global LAST_RESULT
    from concourse.bass_utils import run_bass_kernel_spmd

    X0 = np.asarray(X0, dtype=np.float32)
    MA = np.asarray(MA, dtype=np.float32)
    MW = np.asarray(MW, dtype=np.float32)
    bA_z = np.asarray(bA_z, dtype=np.float32)
    bW_z = np.asarray(bW_z, dtype=np.float32)
    by_w = np.asarray(by_w, dtype=np.float32)

    bA = np.float32(0.5) * np.exp(-bA_z[0, 0] * bA_z[0, 0]) + np.float32(0.5)
    bW = np.float32(0.5) * np.exp(-bW_z[0, 0] * bW_z[0, 0]) + np.float32(0.5)
    I = np.eye(N, dtype=np.float32)
    A = (1 - bA) * (MA + MA.T) + bA * (MA - MA.T) - np.float32(YA) * I
    C = (1 - bA) * (MW + MW.T) + bW * (MW - MW.T) - np.float32(YW) * I

    f16 = lambda x: x.astype(np.float16).astype(np.float32)
    Ah = f16(np.float32(STEP) * A)
    Cs = f16(np.float32(STEP) * C)
    Dm = I + np.float32(STEP) * A
    CsD = f16((np.float32(STEP) * C) @ Dm)
    CsD2 = f16((np.float32(STEP) * C) @ Dm @ Dm)
    byh = f16(by_w)

    NPACK = 19 * 128 + BLOC + 4 * BLOC
    base = np.zeros((128, NPACK), dtype=np.float16)
    for wi, M in enumerate([Ah, Cs, CsD, CsD2]):
        MT = M.T.astype(np.float16)
        for k in range(2):
            for m in range(2):
                idx = 4 * wi + 2 * k + m
                base[:, idx * 128:(idx + 1) * 128] = \
                    MT[128 * k:128 * (k + 1), 128 * m:128 * (m + 1)]
    base[0, 16 * 128:16 * 128 + 128] = byh[0:128, 0].astype(np.float16)
    base[0, 17 * 128:17 * 128 + 128] = byh[128:256, 0].astype(np.float16)
    base[:, 18 * 128:19 * 128] = np.eye(128, dtype=np.float16)
    base[0, 2432:2432 + BLOC] = 1.0

    in_maps = []
    for i in range(NCORES):
        Xc = X0[i * BLOC:(i + 1) * BLOC, :] / np.float32(STEP)   # [32, 256]
        z0 = Xc.T.astype(np.float16)                              # [256, 32]
        z0f = z0.astype(np.float32)
        T0f = np.tanh(Cs @ z0f + byh).astype(np.float16)          # [256, 32]
        WPACKh = base.copy()
        for c in range(2):
            WPACKh[:, 2464 + c * BLOC:2464 + (c + 1) * BLOC] = \
                z0[c * 128:(c + 1) * 128, :]
            WPACKh[:, 2528 + c * BLOC:2528 + (c + 1) * BLOC] = \
                T0f[c * 128:(c + 1) * 128, :]
        in_maps.append({"WPACK": WPACKh})

    nc = _build(NSTEPS)
    res = run_bass_kernel_spmd(nc, in_maps, core_ids=list(range(NCORES)))
    LAST_RESULT = res

    out = np.empty((BS, TMAX, N), dtype=np.float32)
    out[:, 0, :] = X0
    for i in range(NCORES):
        O = np.asarray(res.results[i]["OUT"]).reshape(128, NSTEPS, 2, BLOC)
        # [p, t, c, b] -> [b, t, c, p] -> [32, 511, 256]
        blockX = O.transpose(3, 1, 2, 0).reshape(BLOC, NSTEPS, N)
        out[i * BLOC:(i + 1) * BLOC, 1:, :] = (
            blockX.astype(np.float32) * np.float32(STEP))
    return out


if __name__ == "__main__":
    rng = np.random.default_rng(0)
    inputs = {
        "X0": rng.standard_normal((BS, N), dtype=np.float32),
        "MA": rng.standard_normal((N, N), dtype=np.float32) / 16,
        "MW": rng.standard_normal((N, N), dtype=np.float32) / 16,
        "bA_z": np.full((1, 1), 0.65, dtype=np.float32),
        "bW_z": np.full((1, 1), 0.65, dtype=np.float32),
        "by_w": rng.standard_normal((N, 1), dtype=np.float32) / 100,
    }
    out = kernel(**inputs)
    print("out", out.shape, out.dtype, np.abs(out).max())
